# revision 43
# baseline (speedup 1.0000x reference)
"""BCJR decoder (rate-1/2 conv code, 64 states) on 8 Trainium2 cores.

Strategy
--------
Data-parallel over batch: 32 codewords per core. Within a core, each
codeword's T=2048 trellis steps are split into C=16 chunks of 128 steps,
decoded in parallel with L=12 warm-up steps on each side (windowed BCJR).
The time axis is padded with llr_a=+8 "pilot" steps which deterministically
collapse the state to 0, making chunk 0 / chunk 15 boundary conditions exact.

Layout: 128 SBUF partitions = 32 codewords x 4 chunk-groups; 4 more chunks
("f groups") along the free dimension. The interleaved fwd+bwd loop runs
only S+L = 140 iterations: fwd's last ahist write is at t = S+L-1 and bwd's
last bhist write at t = S+L-2, so the remaining warm-down steps are skipped.

Per step: PE matmul (bf16 sign-table x llr triple) builds branch-metric
exponents E in PSUM (two steps batched per buffer); ScalarE does ONE
exp(0.5 E) per 2-step pair for each chain (the per-instruction SBUF-access
overhead on ACT is ~230 ns, so pairing halves it); VectorE does the
alpha/beta gather-mults; the pairwise adds are split GPSIMD/DVE per chain
(PSF_F/PSF_B f-chunks on Pool, rest on DVE — tuned against the cost model).
Normalization runs every NORM_EVERY=32 steps with a stale (2-step-old)
denominator so the reduce+reciprocal sit off the serial path; the bwd
phase is offset (NORM_PHASE_B) so the prenorm alpha/beta peaks never
coincide in the jm product (bf16 overflow guard — NE=48 aligned NaNs).

Epilogue: jm = ahist*bhist and the per-(f,b) m-sum tree. The last NPOOL=2
blocks' jm+first tree level run on GPSIMD (pinned after the loop's final
pairsums — Pool is in-order, so an early big jm would block the recursion);
DVE does the rest, with the LLR transform of the first k-half emitted
mid-epilogue so its Ln overlaps the remaining tree work.
"""

import os
from contextlib import ExitStack

import numpy as np
import ml_dtypes

import concourse.bass as bass
import concourse.mybir as mybir
from concourse import tile as tile_mod
from concourse.tile_rust import add_dep_helper
from concourse.bass_utils import run_bass_kernel_spmd

# ---------------- problem constants (hardcoded) ----------------
B_FULL, N_FULL = 256, 4096
T = N_FULL // 2            # 2048 trellis steps
N_CORES = 8
B_CORE = B_FULL // N_CORES  # 32 codewords per core
C = 16                     # time chunks per codeword
S = T // C                 # 128 steps per chunk
L = 12                     # warmup steps each side
TL = S + 2 * L             # 160 local steps
CF = 4                     # chunks in free dim (C = 4 partition-groups * CF)
PAD_A = 8.0                # llr_a pad value (forces state collapse)
NORM_EVERY = 32
NORM_PHASE_B = 16        # bwd normalize phase: offset so prenorm peaks of
                         # alpha/beta never coincide in the jm product
PSF_F = 2                  # fwd pairsum: f-chunks 0..PSF_F-1 on GPSIMD, rest DVE
PSF_B = 2                  # bwd pairsum: f-chunks 0..PSF_B-1 on GPSIMD, rest DVE

F32 = mybir.dt.float32
BF16 = mybir.dt.bfloat16


def _sign_table():
    """[3, 128] rows (la, l0, l1) x cols (b, s): E[s,b] = sum_c sign[c,(b,s)] * llr_c."""
    gen = ("1111001", "1011011")
    mu = 6
    g = np.array([[int(c) for c in p] for p in gen])
    opf = np.zeros((64, 2), np.int32)
    for s in range(64):
        rbits = [(s >> (mu - 1 - j)) & 1 for j in range(mu)]
        for b in range(2):
            w = np.array([b] + rbits)
            obits = (g @ w) % 2
            opf[s, b] = obits[0] * 2 + obits[1]
    ops = (1.0 - 2.0 * np.array([[(o >> (1 - j)) & 1 for j in range(2)]
                                 for o in range(4)])).astype(np.float32)
    sa = np.concatenate([np.ones(64), -np.ones(64)])
    s0 = np.concatenate([ops[opf[:, 0], 0], ops[opf[:, 1], 0]])
    s1 = np.concatenate([ops[opf[:, 0], 1], ops[opf[:, 1], 1]])
    return np.stack([sa, s0, s1]).astype(np.float32)  # [3, 128]


SIGN_NP = _sign_table()
# block-diag [12, 512]: rows (f*3+c), cols (f', (b,s))
SIGN_BD = np.zeros((12, 512), np.float32)
for _f in range(4):
    SIGN_BD[_f * 3:_f * 3 + 3, _f * 128:(_f + 1) * 128] = SIGN_NP
SIGN_BD = SIGN_BD.astype(ml_dtypes.bfloat16)

# ---------------- bass program ----------------
_NC_CACHE = {}

W = 19                 # llr_t streaming window (steps)
NW = TL // W           # 8 windows


def _ap(a, offset_extra, dims):
    """Custom AP over the same tensor as `a` (partition dim kept)."""
    return bass.AP(tensor=a.tensor, offset=a.offset + offset_extra,
                   ap=[list(a.ap[0])] + [list(d) for d in dims])


def build_nc():
    nc = bass.Bass()
    llr_t_d = nc.declare_dram_parameter("llr_t", [12, TL * 128], BF16, isOutput=False)
    sign_d = nc.declare_dram_parameter("sign", [12, 512], BF16, isOutput=False)
    out_d = nc.declare_dram_parameter("llr_out", [B_CORE, T], F32, isOutput=True)
    dbg = os.environ.get("KDBG", "0") == "1"
    if dbg:
        dbg_jsum = nc.declare_dram_parameter("dbg_jsum", [128, S * 8], F32, isOutput=True)
        dbg_ah = nc.declare_dram_parameter("dbg_ah", [128, 512], BF16, isOutput=True)
        dbg_bh = nc.declare_dram_parameter("dbg_bh", [128, 512], BF16, isOutput=True)
        dbg_jm = nc.declare_dram_parameter("dbg_jm", [128, 512], BF16, isOutput=True)

    mult = mybir.AluOpType.mult
    add = mybir.AluOpType.add

    with tile_mod.TileContext(nc) as tc, ExitStack() as ctx:
        # static ring buffers (pool alloc/release deps would exceed the
        # 1-sync-wait-per-instruction hardware limit)
        def ring(nm, n, shape, dt=F32):
            return [ctx.enter_context(nc.sbuf_tensor(f"{nm}{i}", shape, dt))
                    for i in range(n)]

        e_pb = [ctx.enter_context(nc.psum_tensor(f"epb{_i}", [128, 1024], F32))
                for _i in range(2)]
        e_pf = [ctx.enter_context(nc.psum_tensor(f"epf{_i}", [128, 1024], F32))
                for _i in range(2)]
        g_pb = ring("gpb", 4, [128, 1024], BF16)
        g_pf = ring("gpf", 3, [128, 1024], BF16)
        ag_bufs = ring("agbuf", 4, [128, 512], BF16)
        aw_bufs = ring("awbuf", 6, [128, 256], BF16)
        nrm_bufs = ring("nrmbuf", 2, [128, 2 * CF])
        lt_bufs = ring("ltbuf", 4, [12, W * 128], BF16)
        jm_bufs = ring("jmblk", 2, [128, 16 * 256], BF16)
        tr_bufs = ring("trbuf", 2, [128, 2048], BF16)

        dve_scr = ctx.enter_context(nc.sbuf_tensor("dvescr", [1, 37], F32))
        act_scr = ctx.enter_context(nc.sbuf_tensor("actscr", [1, 8], F32))
        act_scr2 = ctx.enter_context(nc.sbuf_tensor("actscr2", [1, 8], F32))
        # fwd pair-exp absorbers: unique column per pair (a reused slot's
        # WAW wait gets spilled by tile onto the next engine instruction)
        act_scr3 = ctx.enter_context(nc.sbuf_tensor("actscr3", [1, 180], F32))
        act_scr4 = ctx.enter_context(nc.sbuf_tensor("actscr4", [1, 180], F32))
        sign_t = ctx.enter_context(nc.sbuf_tensor("sign_sb", [12, 512], BF16))
        sign_sb = sign_t[:]
        nc.sync.dma_start(out=sign_sb, in_=sign_d[:])

        state = {"prev_g": None, "gstep": 0, "d1_bufs": [None] * 16,
                 "nb": 0, "nf": 0, "nbw": 0, "env": None}
        ahist_t = ctx.enter_context(nc.sbuf_tensor("ahist", [128, S * 256], BF16))
        ahist = ahist_t[:]
        bhist_t = ctx.enter_context(nc.sbuf_tensor("bhist", [128, S * 256], BF16))
        bhist = bhist_t[:]
        jsum_t = ctx.enter_context(nc.sbuf_tensor("jsum", [128, S * 8], F32))
        jsum = jsum_t[:]
        _counters = {"g": 0, "ag": 0, "aw": 0, "nrm": 0, "lt": 0, "jm": 0, "tr": 0}

        def nxt(nm, bufs):
            i = _counters[nm]
            _counters[nm] = i + 1
            return bufs[i % len(bufs)]

        def emit_mm_b(tau, lt_sb, fresh_dma):
            """Bwd: matmul for step tau into half of a PSUM pair; one
            deinterleaved exp per pair (runs 2 steps ahead of the recursion).

            PE Matmult (LW struct) supports only ONE sync wait, so 1-element
            dummy matmuls absorb the PSUM-WAR and window-DMA waits first.
            """
            n = state["nbw"]
            state["nbw"] = n + 1
            pidx, half = divmod(n, 2)
            e_pair = e_pb[pidx % 2]
            col = (tau % W) * 128
            off = half * 512
            nc.tensor.matmul(out=e_pair[0:1, off:off + 1],
                             lhsT=sign_t[0:1, 0:1],
                             rhs=sign_t[0:1, 0:1], start=True, stop=True)
            if fresh_dma:
                nc.tensor.matmul(out=e_pair[0:1, off:off + 1],
                                 lhsT=lt_sb[0:1, col:col + 1],
                                 rhs=sign_t[0:1, 0:1], start=True, stop=True)
            nc.tensor.matmul(
                out=e_pair[:, off:off + 512], lhsT=lt_sb[:, col:col + 128],
                rhs=sign_sb, start=True, stop=True)
            if half == 1:
                gpb = g_pb[pidx % 4][:]
                # absorber chain: anchor (ACT self-progress via previous bwd
                # pair), c2a (DVE progress covering the g_pb WAR), then exp
                # carries only the PE wait. See emit_mm_f.
                a_src = g_pb[(pidx - 1) % 4][:] if pidx >= 1 else sign_sb
                i_anc = nc.scalar.copy(out=act_scr4[0:1, pidx:pidx + 1],
                                       in_=a_src[0:1, 0:1])
                g = state["gstep"]
                if g >= 2 and state["d1_bufs"][(g - 2) % 16] is not None:
                    dsrc = state["d1_bufs"][(g - 2) % 16]
                else:
                    dsrc = sign_sb[0:1, 0:1]
                i_c2a = nc.scalar.copy(out=act_scr4[0:1, 90 + pidx:91 + pidx],
                                       in_=dsrc)
                add_dep_helper(i_c2a.ins, i_anc.ins, False, "act-order")
                # both halves deinterleaved: within half h (cols h*512..),
                # col (f, b, k, i) = 128f + 64b + k + 32i  <-  E (f, b, s=2k+i)
                i_exp = nc.scalar.activation(
                    out=_ap(gpb, 0, [[128, 8], [64, 2], [1, 32], [32, 2]]),
                    in_=_ap(e_pair[:], 0, [[128, 8], [64, 2], [2, 32], [1, 2]]),
                    func=mybir.ActivationFunctionType.Exp, scale=0.5)
                add_dep_helper(i_exp.ins, i_c2a.ins, False, "act-order")

        def emit_mm_f(tau, lt_sb, fresh_dma):
            """Fwd: matmul for step tau into half of a PSUM pair; one exp
            per pair (runs 2 steps ahead of the recursion)."""
            n = state["nf"]
            state["nf"] = n + 1
            pidx, half = divmod(n, 2)
            e_pair = e_pf[pidx % 2]
            col = (tau % W) * 128
            off = half * 512
            nc.tensor.matmul(out=e_pair[0:1, off:off + 1],
                             lhsT=sign_t[0:1, 0:1],
                             rhs=sign_t[0:1, 0:1], start=True, stop=True)
            if fresh_dma:
                nc.tensor.matmul(out=e_pair[0:1, off:off + 1],
                                 lhsT=lt_sb[0:1, col:col + 1],
                                 rhs=sign_t[0:1, 0:1], start=True, stop=True)
            nc.tensor.matmul(
                out=e_pair[:, off:off + 512], lhsT=lt_sb[:, col:col + 128],
                rhs=sign_sb, start=True, stop=True)
            if half == 1:
                gp = g_pf[pidx % 3][:]
                # anchor: ACT-self wait on the PREVIOUS fwd pair's exp (fresh
                # waited_max so stale WAW/WAR self-waits prune; own slot would
                # create an unprunable WAR of the exp on its own anchor)
                a_src = g_pf[(pidx - 1) % 3][:] if pidx >= 1 else sign_sb
                i_anc = nc.scalar.copy(out=act_scr3[0:1, pidx:pidx + 1],
                                       in_=a_src[0:1, 0:1])
                g = state["gstep"]
                if g >= 2 and state["d1_bufs"][(g - 2) % 16] is not None:
                    dsrc = state["d1_bufs"][(g - 2) % 16]
                else:
                    dsrc = sign_sb[0:1, 0:1]
                i_c2a = nc.scalar.copy(out=act_scr3[0:1, 90 + pidx:91 + pidx],
                                       in_=dsrc)
                add_dep_helper(i_c2a.ins, i_anc.ins, False, "act-order")
                i_exp = nc.scalar.activation(
                    out=_ap(gp, 0, [[128, 8], [64, 2], [1, 64]]),
                    in_=_ap(e_pair[:], 0, [[128, 8], [64, 2], [1, 64]]),
                    func=mybir.ActivationFunctionType.Exp, scale=0.5)
                add_dep_helper(i_exp.ins, i_c2a.ins, False, "act-order")

        def mark_d1(i_d1_src):
            """Record the dve_scr slot source for step g (for c2a 4 steps on)."""
            g = state["gstep"] - 1
            state["d1_bufs"][g % 16] = dve_scr[0:1, g % 16:g % 16 + 1]

        def load_window(w):
            lt_sb = nxt("lt", lt_bufs)
            nc.sync.dma_start(out=lt_sb[:],
                              in_=llr_t_d[:, w * W * 128:(w + 1) * W * 128])
            return lt_sb

        def normalize(cur, old=None):
            """Rescale cur by 1/sum(old); old defaults to cur. A stale old
            (the chain's state from 2 steps back) lets the reduce+recip run
            off the critical path -- any positive per-(row,f) scale is valid,
            it only guards bf16 range and cancels in the final LLR ratio."""
            if old is None:
                old = cur
            nb = nxt("nrm", nrm_bufs)
            asum = nb[:, 0:CF]
            # absorber: old has a GPSIMD writer and a DVE writer;
            # i_n carries the Pool wait so the reduce keeps only its DVE wait
            i_n = nc.vector.tensor_copy(out=dve_scr[0:1, 32:33],
                                        in_=old[0:1, 0:1])
            i_rd = nc.vector.tensor_reduce(
                out=asum, in_=old.rearrange("p (f s) -> p f s", f=CF),
                axis=mybir.AxisListType.X, op=add)
            add_dep_helper(i_rd.ins, i_n.ins, False, "dve-order")
            rz = nb[:, CF:2 * CF]
            nc.vector.reciprocal(out=rz, in_=asum)
            anorm = nxt("aw", aw_bufs)[:]
            rz_b = _ap(rz, 0, [[1, CF], [0, 64]])
            # absorber: cur is fresh (Pool + DVE writers); i_n2 carries the
            # Pool wait so the apply-mult keeps only its DVE wait
            i_n2 = nc.vector.tensor_copy(out=dve_scr[0:1, 34:35],
                                         in_=cur[0:1, 0:1])
            add_dep_helper(i_n2.ins, i_rd.ins, False, "dve-order")
            i_ap2 = nc.vector.tensor_tensor(
                out=anorm.rearrange("p (f s) -> p f s", f=CF),
                in0=cur.rearrange("p (f s) -> p f s", f=CF),
                in1=rz_b, op=mult)
            add_dep_helper(i_ap2.ins, i_n2.ins, False, "dve-order")
            return anorm

        # ------------- interleaved forward + backward -------------
        # Two independent recursion chains share each engine; while one
        # chain's DVE self-semaphore propagates, the other chain's ops
        # execute, so the period is engine-busy-bound, not latency-bound.
        def fwd_step(tau, env):
            g = state["gstep"]
            state["gstep"] = g + 1
            gp = g_pf[(tau // 2) % 3][:]
            goff = (tau % 2) * 512
            alpha = env["alpha"]
            ag = nxt("ag", ag_bufs)[:]
            # DVE absorbers: i_d1 carries the Pool wait (alpha f0-2 part),
            # i_d2 the DVE self-wait (alpha f3 part); mult keeps only ACT
            i_d1 = nc.vector.tensor_copy(
                out=dve_scr[0:1, g % 16:g % 16 + 1], in_=alpha[0:1, 0:1])
            mark_d1(i_d1)
            i_d2 = nc.vector.tensor_copy(
                out=dve_scr[0:1, 16 + g % 16:17 + g % 16],
                in_=alpha[0:1, 64 * PSF_F:64 * PSF_F + 1])
            add_dep_helper(i_d2.ins, i_d1.ins, False, "dve-order")
            a_b = _ap(alpha, 0, [[64, CF], [0, 2], [1, 64]])
            i_ag = nc.vector.tensor_tensor(
                out=ag.rearrange("p (f b s) -> p f b s", f=CF, b=2),
                in0=_ap(gp, goff, [[128, CF], [64, 2], [1, 64]]),
                in1=a_b, op=mult)
            add_dep_helper(i_ag.ins, i_d2.ins, False, "dve-order")
            # pairsum -> alpha' (prenorm); store to ahist when in output range
            if L <= tau < L + S:
                dst = ahist[:, (tau - L) * 256:(tau - L + 1) * 256]
            else:
                dst = nxt("aw", aw_bufs)[:]
            # split: f 0..PSF_F-1 on GPSIMD, rest on DVE
            i_pp = nc.gpsimd.tensor_tensor(
                out=_ap(dst, 0, [[64, PSF_F], [32, 2], [1, 32]]),
                in0=_ap(ag, 0, [[128, PSF_F], [64, 2], [2, 32]]),
                in1=_ap(ag, 1, [[128, PSF_F], [64, 2], [2, 32]]),
                op=add)
            state["last_pool_ins_f"] = i_pp
            state["last_pool_dst"] = dst
            if CF > PSF_F:
                nc.vector.tensor_tensor(
                    out=_ap(dst, 64 * PSF_F,
                            [[64, CF - PSF_F], [32, 2], [1, 32]]),
                    in0=_ap(ag, 128 * PSF_F,
                            [[128, CF - PSF_F], [64, 2], [2, 32]]),
                    in1=_ap(ag, 128 * PSF_F + 1,
                            [[128, CF - PSF_F], [64, 2], [2, 32]]),
                    op=add)
            alpha = dst
            if tau % NORM_EVERY == NORM_EVERY - 1:
                alpha = normalize(alpha, env.get("alpha_p2"))
            env["alpha_p2"] = env.get("alpha_p1")
            env["alpha_p1"] = dst
            env["alpha"] = alpha

        def bwd_step(t, tau, env):
            g = state["gstep"]
            state["gstep"] = g + 1
            gpb = g_pb[(t // 2) % 4][:]
            goff = (t % 2) * 512
            beta = env["beta"]
            bg = nxt("ag", ag_bufs)[:]
            i_d1 = nc.vector.tensor_copy(
                out=dve_scr[0:1, g % 16:g % 16 + 1], in_=beta[0:1, 0:1])
            mark_d1(i_d1)
            i_d2 = nc.vector.tensor_copy(
                out=dve_scr[0:1, 16 + g % 16:17 + g % 16],
                in_=beta[0:1, 64 * PSF_B:64 * PSF_B + 1])
            add_dep_helper(i_d2.ins, i_d1.ins, False, "dve-order")
            # layout (f, b, m, k): all operands unit-stride innermost (2x mode)
            b_g = _ap(beta, 0, [[64, CF], [32, 2], [0, 2], [1, 32]])
            g_in = _ap(gpb, goff, [[128, CF], [64, 2], [32, 2], [1, 32]])
            bg_out = _ap(bg, 0, [[128, CF], [64, 2], [32, 2], [1, 32]])
            i_bg = nc.vector.tensor_tensor(out=bg_out, in0=g_in, in1=b_g,
                                           op=mult)
            add_dep_helper(i_bg.ins, i_d2.ins, False, "dve-order")
            # beta for step tau goes to bhist[tau-1-L] (jm_k pairs with
            # beta_{k+1}, the beta bg uses at step k+L)
            kb = tau - 1 - L
            if 0 <= kb < S:
                dst = bhist[:, kb * 256:(kb + 1) * 256]
            else:
                dst = nxt("aw", aw_bufs)[:]
            # iterate (f, m, k): out idx 64f + 2k + m ; bg idx 128f + b64 + 32m + k
            i_pp = nc.gpsimd.tensor_tensor(
                out=_ap(dst, 0, [[64, PSF_B], [1, 2], [2, 32]]),
                in0=_ap(bg, 0, [[128, PSF_B], [32, 2], [1, 32]]),
                in1=_ap(bg, 64, [[128, PSF_B], [32, 2], [1, 32]]),
                op=add)
            state["last_pool_ins_b"] = i_pp
            state["last_pool_dst"] = dst
            nc.vector.tensor_tensor(
                out=_ap(dst, 64 * PSF_B,
                        [[64, CF - PSF_B], [1, 2], [2, 32]]),
                in0=_ap(bg, 128 * PSF_B,
                        [[128, CF - PSF_B], [32, 2], [1, 32]]),
                in1=_ap(bg, 128 * PSF_B + 64,
                        [[128, CF - PSF_B], [32, 2], [1, 32]]),
                op=add)
            beta = dst
            if tau % NORM_EVERY == NORM_PHASE_B:
                # stale-scale only once past warmup (aw ring too short before)
                old = env.get("beta_p2") if t > 14 else None
                beta = normalize(beta, old)
            env["beta_p2"] = env.get("beta_p1")
            env["beta_p1"] = dst
            env["beta"] = beta

        env = {}
        state["env"] = env
        env["alpha"] = nxt("aw", aw_bufs)[:]
        nc.vector.memset(env["alpha"], 1.0 / 64)
        env["beta"] = nxt("aw", aw_bufs)[:]
        nc.vector.memset(env["beta"], 1.0 / 64)
        # matmul emission leads each recursion by 2 steps so each pair-exp
        # completes before the first multiply that reads it
        ltf_sb = load_window(0)
        emit_mm_f(0, ltf_sb, True)
        emit_mm_f(1, ltf_sb, False)
        ltb_sb = load_window((TL - 1) // W)
        emit_mm_b(TL - 1, ltb_sb, True)
        emit_mm_b(TL - 2, ltb_sb, False)
        # Useful work ends at t = TLOOP-1 = S+L-1: fwd's last ahist write is
        # at t = L+S-1; bwd's last bhist write (kb=0) is at t = TLOOP-2.
        # Iterations beyond that only decay warm-down state nobody reads.
        TLOOP = S + L
        for t in range(TLOOP):
            tau_b = TL - 1 - t
            if t % W == W - 2 and t < TLOOP - 2:
                ltf_sb = load_window((t + 2) // W)
            if t < TLOOP - 2:
                emit_mm_f(t + 2, ltf_sb, t % W == W - 2)
            tau_e = tau_b - 2
            if t < TLOOP - 2:
                fresh_e = tau_e % W == W - 1
                if fresh_e:
                    ltb_sb = load_window(tau_e // W)
                emit_mm_b(tau_e, ltb_sb, fresh_e)
            fwd_step(t, env)
            if t <= TLOOP - 2:
                bwd_step(t, tau_b, env)

        # ---------------- epilogue: jm = ahist*bhist, half-sum tree --------
        # Split across engines: Pool (idle post-loop) takes jm+first tree
        # level of the last NPOOL blocks; DVE does the rest and all tails.
        # absorber: one explicit wait on the final GPSIMD pairsum covers all
        # Pool-side writes of ahist/bhist, so each jm keeps only its DVE wait
        NPOOL = 2
        NBLK = S // 16
        tr_pool_t = ctx.enter_context(nc.sbuf_tensor("trpool", [128, 2048], BF16))

        def tree_tail(cur, width, blk, eng_first=None):
            """DVE halving tree from `width` down to the fp32 jsum write."""
            first = True
            while width > 2:
                half = width // 2
                t_out = nxt("tr", tr_bufs)[:]
                i0 = _ap(cur, 0, [[width, 128], [1, half]])
                i1 = _ap(cur, half, [[width, 128], [1, half]])
                i_t = nc.vector.tensor_tensor(
                    out=_ap(t_out, 0, [[half, 128], [1, half]]),
                    in0=i0, in1=i1, op=add)
                if first and eng_first is not None:
                    add_dep_helper(i_t.ins, eng_first.ins, False, "dve-order")
                first = False
                cur = t_out
                width = half
            # final level: 2 -> 1, fp32 out into jsum (cols k*8 + f*2 + b)
            i0 = _ap(cur, 0, [[2, 128]])
            i1 = _ap(cur, 1, [[2, 128]])
            nc.vector.tensor_tensor(
                out=_ap(jsum, blk * 128, [[1, 128]]),
                in0=i0, in1=i1, op=add)

        # Pool handles jm+L1 of the last NPOOL blocks; both L1s write the
        # same tr_pool (a full L1 output is 2048 dense cols). The DVE tail of
        # the first Pool block is emitted between the two Pool blocks so
        # tile's WAR sem makes L1(second) wait for its read.
        def pool_block(blk, prev_pool):
            base = blk * 16 * 256
            jm = jm_bufs[1][:]
            i_pjm = nc.gpsimd.tensor_tensor(
                out=jm, in0=ahist[:, base:base + 4096],
                in1=bhist[:, base:base + 4096], op=mult)
            if prev_pool is None:
                add_dep_helper(i_pjm.ins, state["last_pool_ins_f"].ins, False,
                               "pool-order")
                add_dep_helper(i_pjm.ins, state["last_pool_ins_b"].ins, False,
                               "pool-order")
            else:
                add_dep_helper(i_pjm.ins, prev_pool.ins, False, "pool-order")
            i_l1 = nc.gpsimd.tensor_tensor(
                out=_ap(tr_pool_t[:], 0, [[16, 128], [1, 16]]),
                in0=_ap(jm, 0, [[32, 128], [1, 16]]),
                in1=_ap(jm, 16, [[32, 128], [1, 16]]),
                op=add)
            return i_l1

        def pool_tail(blk, j, prev_dve):
            # absorber: i_pt carries the Pool wait (that block's L1 write) so
            # the first tree level keeps only its DVE-self wait
            i_pt = nc.vector.tensor_copy(
                out=dve_scr[0:1, 35 + j:36 + j], in_=tr_pool_t[0:1, 0:1])
            add_dep_helper(i_pt.ins, prev_dve.ins, False, "dve-order")
            cur = _ap(tr_pool_t[:], 0, [[1, 2048]])
            tree_tail(cur, 16, blk, eng_first=i_pt)
            return i_pt

        # ---------------- epilogue: llr = ln(j0 / j1), split in k-halves ---
        # ratio first: j0/j1 = exp(llr) stays in the ACT Ln table's valid
        # input range, while raw jsum values (prenorm products) can reach
        # e^70 and fall off the table. Half A (k 0..63, jm blocks 0-3) is
        # emitted mid-epilogue so its Ln + store DMA overlap the remaining
        # blocks; half B finishes after the Pool-block tails.
        rat_t = ctx.enter_context(nc.sbuf_tensor("ratbuf", [128, 512], F32))
        rat = rat_t[:]
        llr_t2 = ctx.enter_context(nc.sbuf_tensor("llrsb", [128, 512], F32))
        llr_sb = llr_t2

        def llr_half(h):
            rcp = llr_sb[:]  # scratch for 1/j1 before Ln overwrites it
            in0 = _ap(jsum, h * 512, [[2, CF], [8, S // 2]])
            in1 = _ap(jsum, h * 512 + 1, [[2, CF], [8, S // 2]])
            rcp_h = _ap(rcp, h * 64, [[128, CF], [1, S // 2]])
            rat_h = _ap(rat, h * 64, [[128, CF], [1, S // 2]])
            nc.vector.reciprocal(out=rcp_h, in_=in1)
            nc.vector.tensor_tensor(out=rat_h, in0=in0, in1=rcp_h, op=mult)
            nc.scalar.activation(out=_ap(llr_sb[:], h * 64,
                                         [[128, CF], [1, S // 2]]),
                                 in_=rat_h,
                                 func=mybir.ActivationFunctionType.Ln)
            if h == 1:
                # single fused store (two DMAs would land on two HW queues
                # and the exit drain can carry only one sem wait)
                src_ap = llr_sb[:].rearrange("p (f k) -> p f k", f=4)
                dst_ap = bass.AP(tensor=out_d[:].tensor, offset=0,
                                 ap=[[2048, 32], [512, 4], [128, 4], [1, 128]])
                nc.sync.dma_start(out=dst_ap, in_=src_ap)

        i_l1a = pool_block(NBLK - 2, None)

        # blocks of 16 k-steps: jm [128, 16*256]; cols k(16) f(4) b(2) m(32)
        # bhist[0] (bwd t=TLOOP-2) is the last-scheduled Pool history write;
        # Pool is in-order so waiting on it covers the fwd one too
        i_ep = nc.vector.tensor_copy(out=dve_scr[0:1, 33:34],
                                     in_=bhist[0:1, 0:1])
        prev_ep = i_ep
        for blk in range(NBLK - NPOOL):
            base = blk * 16 * 256
            jm = jm_bufs[0][:]
            i_jm = nc.vector.tensor_tensor(
                out=jm, in0=ahist[:, base:base + 4096],
                in1=bhist[:, base:base + 4096], op=mult)
            add_dep_helper(i_jm.ins, prev_ep.ins, False, "dve-order")
            prev_ep = i_jm
            tree_tail(jm, 32, blk)
            if blk == 3:
                # tail of the first Pool block, early enough that the second
                # Pool block's L1 never stalls on its tr_pool read
                prev_ep = pool_tail(NBLK - 2, 0, prev_ep)
                llr_half(0)
        i_l1b = pool_block(NBLK - 1, i_l1a)
        pool_tail(NBLK - 1, 1, prev_ep)
        llr_half(1)

        if dbg:
            nc.sync.dma_start(out=dbg_jsum[:], in_=jsum)
            nc.sync.dma_start(out=dbg_ah[:, 0:256], in_=ahist[:, 0:256])
            nc.sync.dma_start(out=dbg_ah[:, 256:512], in_=ahist[:, 64*256:64*256+256])
            nc.sync.dma_start(out=dbg_bh[:, 0:256], in_=bhist[:, 0:256])
            nc.sync.dma_start(out=dbg_bh[:, 256:512], in_=bhist[:, 64*256:64*256+256])
            nc.sync.dma_start(out=dbg_jm[:], in_=jm_bufs[0][:][:, 0:512])

    return nc


_ENG_SELF = {"PE": "PE_", "DVE": "DVE_", "Activation": "Activation_",
             "Pool": "Pool_", "SP": "SP_"}


def _prune_waits(nc):
    """Drop sem waits already implied, so each instruction carries <=1.

    HW structs accept one sync wait per instruction. Tile emits waits that
    are provably satisfied at issue. Vector-clock rules:
      - cross-engine sems: knowledge from transitive joins of kept waits
      - self sems (same engine): only monotone vs explicitly-waited values
        (ACT/DVE completion is not implied by issue order); PE and DMA
        queues complete in order, so own-increment knowledge counts there.
    """
    know = {}        # proc -> {sem_id: known completed value}
    waited_max = {}  # proc -> {sem_id: max explicitly waited}
    sem_total = {}   # sem_id -> running total
    hist = {}        # sem_id -> [(total_after, snapshot)]
    out_dma_sems = set()
    bad = []
    for b in nc.m.functions[0].blocks:
        for i in b.instructions:
            si = i.sync_info
            op = str(getattr(i, "opcode", type(i).__name__))
            if si is None:
                continue
            upds = [u for u in (si.on_update or [])
                    if u.sync_type == "semaphore"
                    and u.update_mode in ("sem-inc", "sem-add-imm")]
            if "DMACopy" in op and upds:
                proc = str(upds[0].ant_name)
                outs = getattr(i, "outs", None) or []
                if outs and "llr_out" in str(getattr(outs[0], "memref", "")):
                    out_dma_sems.add(upds[0].id)
            else:
                proc = getattr(i.engine, "value", str(i.engine))
            k = know.setdefault(proc, {})
            wm = waited_max.setdefault(proc, {})
            in_order = (proc == "PE" or proc == "Pool"
                        or proc.startswith("DMAHW"))
            if "Drain" in op and si.on_wait and len(si.on_wait) > 1:
                keep_d = [w for w in si.on_wait if w.id in out_dma_sems]
                # several output DMAs on one queue: keep only the max-value
                # wait per sem (the queue completes in order)
                best = {}
                for w in keep_d:
                    b = best.get(w.id)
                    if b is None or (w.wait_value or 0) > (b.wait_value or 0):
                        best[w.id] = w
                si.on_wait = list(best.values())
                continue
            skip = ("Drain" in op) or ("EventSem" in op)
            ow = list(si.on_wait or [])
            if ow and not skip:
                keep = []
                for w in ow:
                    if (w.sync_type != "semaphore"
                            or w.wait_mode != "sem-ge-imm"
                            or w.wait_value is None
                            or str(w.ant_name).startswith("barrier")):
                        keep.append(w)
                        continue
                    v = w.wait_value
                    nm = str(w.ant_name)
                    is_self = nm == proc or nm.startswith(proc + "_")
                    if is_self:
                        implied = (wm.get(w.id, -1) >= v
                                   or (in_order and k.get(w.id, 0) >= v))
                    else:
                        implied = (k.get(w.id, 0) >= v
                                   or wm.get(w.id, -1) >= v)
                    if implied:
                        continue
                    keep.append(w)
                    wm[w.id] = max(wm.get(w.id, -1), v)
                    for tot, snap in hist.get(w.id, ()):
                        if tot >= v:
                            for s2, v2 in snap.items():
                                if k.get(s2, 0) < v2:
                                    k[s2] = v2
                            break
                    if k.get(w.id, 0) < v:
                        k[w.id] = v
                if len(keep) != len(ow):
                    si.on_wait = keep
                    ow = keep
                if len(ow) > 1:
                    bad.append((i.name, op,
                                [(x.ant_name, x.wait_value) for x in ow]))
            for u in upds:
                tot = sem_total.get(u.id, 0) + (u.update_value or 0)
                sem_total[u.id] = tot
                k[u.id] = tot
                hist.setdefault(u.id, []).append((tot, dict(k)))
    if bad:
        raise RuntimeError(f"{len(bad)} insts still multi-wait: {bad[:8]}")
    return nc


def _get_nc():
    if "nc" not in _NC_CACHE:
        _NC_CACHE["nc"] = _prune_waits(build_nc())
    return _NC_CACHE["nc"]


# ---------------- host-side layout ----------------
def _prep_core(llr_ch_c, llr_a_c):
    """llr_ch_c [32, 4096], llr_a_c [32, 2048] -> llr_t [12, TL*128] bf16."""
    lc = np.zeros((B_CORE, T + 2 * L, 2), np.float32)
    lc[:, L:L + T] = llr_ch_c.reshape(B_CORE, T, 2)
    la = np.full((B_CORE, T + 2 * L), PAD_A, np.float32)
    la[:, L:L + T] = llr_a_c
    # windows [B, C, TL, comp]
    idx = (np.arange(C)[:, None] * S + np.arange(TL)[None, :])  # [C, TL]
    w = np.stack([la[:, idx], lc[:, idx, 0], lc[:, idx, 1]], -1)  # [B, C, TL, 3]
    # chunk c = g*4+f ; row = cw*4+g ; llr_t[f*3+comp, tau*128+row]
    w = w.reshape(B_CORE, 4, 4, TL, 3)            # [cw, g, f, tau, comp]
    w = w.transpose(2, 4, 3, 0, 1)                # [f, comp, tau, cw, g]
    return np.ascontiguousarray(
        w.reshape(12, TL * 128)).astype(ml_dtypes.bfloat16)


def _run(llr_ch, llr_a, trace=False):
    nc = _get_nc()
    in_maps = []
    for core in range(N_CORES):
        sl = slice(core * B_CORE, (core + 1) * B_CORE)
        in_maps.append({
            "llr_t": _prep_core(np.asarray(llr_ch[sl], np.float32),
                                np.asarray(llr_a[sl], np.float32)),
            "sign": SIGN_BD,
        })
    res = run_bass_kernel_spmd(nc, in_maps, core_ids=list(range(N_CORES)),
                               trace=trace)
    out = np.concatenate([r["llr_out"] for r in res.results], 0)
    return out.astype(np.float32), res


def kernel(llr_ch, llr_a):
    out, _ = _run(llr_ch, llr_a, trace=False)
    return out



# revision 53
# speedup vs baseline: 1.0020x; 1.0020x over previous
"""BCJR decoder (rate-1/2 conv code, 64 states) on 8 Trainium2 cores.

Strategy
--------
Data-parallel over batch: 32 codewords per core. Within a core, each
codeword's T=2048 trellis steps are split into C=16 chunks of 128 steps,
decoded in parallel with L=12 warm-up steps on each side (windowed BCJR).
The time axis is padded with llr_a=+8 "pilot" steps which deterministically
collapse the state to 0, making chunk 0 / chunk 15 boundary conditions exact.

Layout: 128 SBUF partitions = 32 codewords x 4 chunk-groups; 4 more chunks
("f groups") along the free dimension. The interleaved fwd+bwd loop runs
only S+L = 140 iterations: fwd's last ahist write is at t = S+L-1 and bwd's
last bhist write at t = S+L-2, so the remaining warm-down steps are skipped.

Per step: PE matmul (bf16 sign-table x llr triple) builds branch-metric
exponents E in PSUM (two steps batched per buffer); ScalarE does ONE
exp(0.5 E) per 2-step pair for each chain (the per-instruction SBUF-access
overhead on ACT is ~230 ns, so pairing halves it); VectorE does the
alpha/beta gather-mults; the pairwise adds are split GPSIMD/DVE per chain
(PSF_F/PSF_B f-chunks on Pool, rest on DVE — tuned against the cost model).
Normalization runs every NORM_EVERY=32 steps with a stale (2-step-old)
denominator so the reduce+reciprocal sit off the serial path; the bwd
phase is offset (NORM_PHASE_B) so the prenorm alpha/beta peaks never
coincide in the jm product (bf16 overflow guard — NE=48 aligned NaNs).

Epilogue: jm = ahist*bhist and the per-(f,b) m-sum tree. The last NPOOL=2
blocks' jm+first tree level run on GPSIMD (pinned after the loop's final
pairsums — Pool is in-order, so an early big jm would block the recursion);
DVE does the rest, with the LLR transform of the first k-half emitted
mid-epilogue so its Ln overlaps the remaining tree work.
"""

import os
from contextlib import ExitStack

import numpy as np
import ml_dtypes

import concourse.bass as bass
import concourse.mybir as mybir
from concourse import tile as tile_mod
from concourse.tile_rust import add_dep_helper
from concourse.bass_utils import run_bass_kernel_spmd

# ---------------- problem constants (hardcoded) ----------------
B_FULL, N_FULL = 256, 4096
T = N_FULL // 2            # 2048 trellis steps
N_CORES = 8
B_CORE = B_FULL // N_CORES  # 32 codewords per core
C = 16                     # time chunks per codeword
S = T // C                 # 128 steps per chunk
L = 12                     # warmup steps each side
TL = S + 2 * L             # 160 local steps
CF = 4                     # chunks in free dim (C = 4 partition-groups * CF)
PAD_A = 8.0                # llr_a pad value (forces state collapse)
NORM_EVERY = 32
NORM_PHASE_B = 16        # bwd normalize phase: offset so prenorm peaks of
                         # alpha/beta never coincide in the jm product
PSF_F = 2                  # fwd pairsum: f-chunks 0..PSF_F-1 on GPSIMD, rest DVE
PSF_B = 2                  # bwd pairsum: f-chunks 0..PSF_B-1 on GPSIMD, rest DVE

F32 = mybir.dt.float32
BF16 = mybir.dt.bfloat16


def _sign_table():
    """[3, 128] rows (la, l0, l1) x cols (b, s): E[s,b] = sum_c sign[c,(b,s)] * llr_c."""
    gen = ("1111001", "1011011")
    mu = 6
    g = np.array([[int(c) for c in p] for p in gen])
    opf = np.zeros((64, 2), np.int32)
    for s in range(64):
        rbits = [(s >> (mu - 1 - j)) & 1 for j in range(mu)]
        for b in range(2):
            w = np.array([b] + rbits)
            obits = (g @ w) % 2
            opf[s, b] = obits[0] * 2 + obits[1]
    ops = (1.0 - 2.0 * np.array([[(o >> (1 - j)) & 1 for j in range(2)]
                                 for o in range(4)])).astype(np.float32)
    sa = np.concatenate([np.ones(64), -np.ones(64)])
    s0 = np.concatenate([ops[opf[:, 0], 0], ops[opf[:, 1], 0]])
    s1 = np.concatenate([ops[opf[:, 0], 1], ops[opf[:, 1], 1]])
    return np.stack([sa, s0, s1]).astype(np.float32)  # [3, 128]


SIGN_NP = _sign_table()
# block-diag [12, 512]: rows (f*3+c), cols (f', (b,s))
SIGN_BD = np.zeros((12, 512), np.float32)
for _f in range(4):
    SIGN_BD[_f * 3:_f * 3 + 3, _f * 128:(_f + 1) * 128] = SIGN_NP
SIGN_BD = SIGN_BD.astype(ml_dtypes.bfloat16)

# ---------------- bass program ----------------
_NC_CACHE = {}

W = 19                 # llr_t streaming window (steps)
NW = TL // W           # 8 windows


def _ap(a, offset_extra, dims):
    """Custom AP over the same tensor as `a` (partition dim kept)."""
    return bass.AP(tensor=a.tensor, offset=a.offset + offset_extra,
                   ap=[list(a.ap[0])] + [list(d) for d in dims])


def build_nc():
    nc = bass.Bass()
    llr_t_d = nc.declare_dram_parameter("llr_t", [12, TL * 128], BF16, isOutput=False)
    sign_d = nc.declare_dram_parameter("sign", [12, 512], BF16, isOutput=False)
    out_d = nc.declare_dram_parameter("llr_out", [B_CORE, T], F32, isOutput=True)
    dbg = os.environ.get("KDBG", "0") == "1"
    if dbg:
        dbg_jsum = nc.declare_dram_parameter("dbg_jsum", [128, S * 8], F32, isOutput=True)
        dbg_ah = nc.declare_dram_parameter("dbg_ah", [128, 512], BF16, isOutput=True)
        dbg_bh = nc.declare_dram_parameter("dbg_bh", [128, 512], BF16, isOutput=True)
        dbg_jm = nc.declare_dram_parameter("dbg_jm", [128, 512], BF16, isOutput=True)

    mult = mybir.AluOpType.mult
    add = mybir.AluOpType.add

    with tile_mod.TileContext(nc) as tc, ExitStack() as ctx:
        # static ring buffers (pool alloc/release deps would exceed the
        # 1-sync-wait-per-instruction hardware limit)
        def ring(nm, n, shape, dt=F32):
            return [ctx.enter_context(nc.sbuf_tensor(f"{nm}{i}", shape, dt))
                    for i in range(n)]

        e_pb = [ctx.enter_context(nc.psum_tensor(f"epb{_i}", [128, 1024], F32))
                for _i in range(2)]
        e_pf = [ctx.enter_context(nc.psum_tensor(f"epf{_i}", [128, 1024], F32))
                for _i in range(2)]
        g_pb = ring("gpb", 4, [128, 1024], BF16)
        g_pf = ring("gpf", 3, [128, 1024], BF16)
        ag_bufs = ring("agbuf", 4, [128, 512], BF16)
        aw_bufs = ring("awbuf", 6, [128, 256], BF16)
        nrm_bufs = ring("nrmbuf", 2, [128, 2 * CF])
        lt_bufs = ring("ltbuf", 4, [12, W * 128], BF16)
        jm_bufs = ring("jmblk", 2, [128, 16 * 256], BF16)
        tr_bufs = ring("trbuf", 2, [128, 2048], BF16)

        dve_scr = ctx.enter_context(nc.sbuf_tensor("dvescr", [1, 37], F32))
        act_scr = ctx.enter_context(nc.sbuf_tensor("actscr", [1, 8], F32))
        act_scr2 = ctx.enter_context(nc.sbuf_tensor("actscr2", [1, 8], F32))
        # fwd pair-exp absorbers: unique column per pair (a reused slot's
        # WAW wait gets spilled by tile onto the next engine instruction)
        act_scr3 = ctx.enter_context(nc.sbuf_tensor("actscr3", [1, 180], F32))
        act_scr4 = ctx.enter_context(nc.sbuf_tensor("actscr4", [1, 180], F32))
        sign_t = ctx.enter_context(nc.sbuf_tensor("sign_sb", [12, 512], BF16))
        sign_sb = sign_t[:]
        nc.gpsimd.dma_start(out=sign_sb, in_=sign_d[:])

        state = {"prev_g": None, "gstep": 0, "d1_bufs": [None] * 16,
                 "nb": 0, "nf": 0, "nbw": 0, "env": None}
        ahist_t = ctx.enter_context(nc.sbuf_tensor("ahist", [128, S * 256], BF16))
        ahist = ahist_t[:]
        bhist_t = ctx.enter_context(nc.sbuf_tensor("bhist", [128, S * 256], BF16))
        bhist = bhist_t[:]
        jsum_t = ctx.enter_context(nc.sbuf_tensor("jsum", [128, S * 8], F32))
        jsum = jsum_t[:]
        _counters = {"g": 0, "ag": 0, "aw": 0, "nrm": 0, "lt": 0, "jm": 0, "tr": 0}

        def nxt(nm, bufs):
            i = _counters[nm]
            _counters[nm] = i + 1
            return bufs[i % len(bufs)]

        def emit_mm_b(tau, lt_sb, fresh_dma):
            """Bwd: matmul for step tau into half of a PSUM pair; one
            deinterleaved exp per pair (runs 2 steps ahead of the recursion).

            PE Matmult (LW struct) supports only ONE sync wait, so 1-element
            dummy matmuls absorb the PSUM-WAR and window-DMA waits first.
            """
            n = state["nbw"]
            state["nbw"] = n + 1
            pidx, half = divmod(n, 2)
            e_pair = e_pb[pidx % 2]
            col = (tau % W) * 128
            off = half * 512
            nc.tensor.matmul(out=e_pair[0:1, off:off + 1],
                             lhsT=sign_t[0:1, 0:1],
                             rhs=sign_t[0:1, 0:1], start=True, stop=True)
            if fresh_dma:
                nc.tensor.matmul(out=e_pair[0:1, off:off + 1],
                                 lhsT=lt_sb[0:1, col:col + 1],
                                 rhs=sign_t[0:1, 0:1], start=True, stop=True)
            nc.tensor.matmul(
                out=e_pair[:, off:off + 512], lhsT=lt_sb[:, col:col + 128],
                rhs=sign_sb, start=True, stop=True)
            if half == 1:
                gpb = g_pb[pidx % 4][:]
                # absorber chain: anchor (ACT self-progress via previous bwd
                # pair), c2a (DVE progress covering the g_pb WAR), then exp
                # carries only the PE wait. See emit_mm_f.
                a_src = g_pb[(pidx - 1) % 4][:] if pidx >= 1 else sign_sb
                i_anc = nc.scalar.copy(out=act_scr4[0:1, pidx:pidx + 1],
                                       in_=a_src[0:1, 0:1])
                g = state["gstep"]
                if g >= 2 and state["d1_bufs"][(g - 2) % 16] is not None:
                    dsrc = state["d1_bufs"][(g - 2) % 16]
                else:
                    dsrc = sign_sb[0:1, 0:1]
                i_c2a = nc.scalar.copy(out=act_scr4[0:1, 90 + pidx:91 + pidx],
                                       in_=dsrc)
                add_dep_helper(i_c2a.ins, i_anc.ins, False, "act-order")
                # both halves deinterleaved: within half h (cols h*512..),
                # col (f, b, k, i) = 128f + 64b + k + 32i  <-  E (f, b, s=2k+i)
                i_exp = nc.scalar.activation(
                    out=_ap(gpb, 0, [[128, 8], [64, 2], [1, 32], [32, 2]]),
                    in_=_ap(e_pair[:], 0, [[128, 8], [64, 2], [2, 32], [1, 2]]),
                    func=mybir.ActivationFunctionType.Exp, scale=0.5)
                add_dep_helper(i_exp.ins, i_c2a.ins, False, "act-order")

        def emit_mm_f(tau, lt_sb, fresh_dma):
            """Fwd: matmul for step tau into half of a PSUM pair; one exp
            per pair (runs 2 steps ahead of the recursion)."""
            n = state["nf"]
            state["nf"] = n + 1
            pidx, half = divmod(n, 2)
            e_pair = e_pf[pidx % 2]
            col = (tau % W) * 128
            off = half * 512
            nc.tensor.matmul(out=e_pair[0:1, off:off + 1],
                             lhsT=sign_t[0:1, 0:1],
                             rhs=sign_t[0:1, 0:1], start=True, stop=True)
            if fresh_dma:
                nc.tensor.matmul(out=e_pair[0:1, off:off + 1],
                                 lhsT=lt_sb[0:1, col:col + 1],
                                 rhs=sign_t[0:1, 0:1], start=True, stop=True)
            nc.tensor.matmul(
                out=e_pair[:, off:off + 512], lhsT=lt_sb[:, col:col + 128],
                rhs=sign_sb, start=True, stop=True)
            if half == 1:
                gp = g_pf[pidx % 3][:]
                # anchor: ACT-self wait on the PREVIOUS fwd pair's exp (fresh
                # waited_max so stale WAW/WAR self-waits prune; own slot would
                # create an unprunable WAR of the exp on its own anchor)
                a_src = g_pf[(pidx - 1) % 3][:] if pidx >= 1 else sign_sb
                i_anc = nc.scalar.copy(out=act_scr3[0:1, pidx:pidx + 1],
                                       in_=a_src[0:1, 0:1])
                g = state["gstep"]
                if g >= 2 and state["d1_bufs"][(g - 2) % 16] is not None:
                    dsrc = state["d1_bufs"][(g - 2) % 16]
                else:
                    dsrc = sign_sb[0:1, 0:1]
                i_c2a = nc.scalar.copy(out=act_scr3[0:1, 90 + pidx:91 + pidx],
                                       in_=dsrc)
                add_dep_helper(i_c2a.ins, i_anc.ins, False, "act-order")
                i_exp = nc.scalar.activation(
                    out=_ap(gp, 0, [[128, 8], [64, 2], [1, 64]]),
                    in_=_ap(e_pair[:], 0, [[128, 8], [64, 2], [1, 64]]),
                    func=mybir.ActivationFunctionType.Exp, scale=0.5)
                add_dep_helper(i_exp.ins, i_c2a.ins, False, "act-order")

        def mark_d1(i_d1_src):
            """Record the dve_scr slot source for step g (for c2a 4 steps on)."""
            g = state["gstep"] - 1
            state["d1_bufs"][g % 16] = dve_scr[0:1, g % 16:g % 16 + 1]

        def load_window(w):
            lt_sb = nxt("lt", lt_bufs)
            nc.sync.dma_start(out=lt_sb[:],
                              in_=llr_t_d[:, w * W * 128:(w + 1) * W * 128])
            return lt_sb

        def norm_prep(cur):
            """Emit reduce+recip on the CURRENT state, two iterations before
            the normalize that uses it. Any positive per-(row,f) scale is
            valid (it only guards bf16 range and cancels in the LLR ratio),
            so the denominator can be two steps stale -- this keeps the
            reduce+recip and their pairsum wait off the apply step's path."""
            nb = nxt("nrm", nrm_bufs)
            asum = nb[:, 0:CF]
            # absorber: cur has a GPSIMD writer and a DVE writer;
            # i_n carries the Pool wait so the reduce keeps only its DVE wait
            i_n = nc.vector.tensor_copy(out=dve_scr[0:1, 32:33],
                                        in_=cur[0:1, 0:1])
            i_rd = nc.vector.tensor_reduce(
                out=asum, in_=cur.rearrange("p (f s) -> p f s", f=CF),
                axis=mybir.AxisListType.X, op=add)
            add_dep_helper(i_rd.ins, i_n.ins, False, "dve-order")
            rz = nb[:, CF:2 * CF]
            nc.vector.reciprocal(out=rz, in_=asum)
            return rz

        def norm_apply(cur, rz):
            anorm = nxt("aw", aw_bufs)[:]
            rz_b = _ap(rz, 0, [[1, CF], [0, 64]])
            # absorber: cur is fresh (Pool + DVE writers); i_n2 carries the
            # Pool wait so the apply-mult keeps only its DVE wait
            i_n2 = nc.vector.tensor_copy(out=dve_scr[0:1, 34:35],
                                         in_=cur[0:1, 0:1])
            i_ap2 = nc.vector.tensor_tensor(
                out=anorm.rearrange("p (f s) -> p f s", f=CF),
                in0=cur.rearrange("p (f s) -> p f s", f=CF),
                in1=rz_b, op=mult)
            add_dep_helper(i_ap2.ins, i_n2.ins, False, "dve-order")
            return anorm

        # ------------- interleaved forward + backward -------------
        # Two independent recursion chains share each engine; while one
        # chain's DVE self-semaphore propagates, the other chain's ops
        # execute, so the period is engine-busy-bound, not latency-bound.
        def fwd_step(tau, env):
            g = state["gstep"]
            state["gstep"] = g + 1
            gp = g_pf[(tau // 2) % 3][:]
            goff = (tau % 2) * 512
            alpha = env["alpha"]
            ag = nxt("ag", ag_bufs)[:]
            # DVE absorbers: i_d1 carries the Pool wait (alpha f0-2 part),
            # i_d2 the DVE self-wait (alpha f3 part); mult keeps only ACT
            i_d1 = nc.vector.tensor_copy(
                out=dve_scr[0:1, g % 16:g % 16 + 1], in_=alpha[0:1, 0:1])
            mark_d1(i_d1)
            i_d2 = nc.vector.tensor_copy(
                out=dve_scr[0:1, 16 + g % 16:17 + g % 16],
                in_=alpha[0:1, 64 * PSF_F:64 * PSF_F + 1])
            add_dep_helper(i_d2.ins, i_d1.ins, False, "dve-order")
            a_b = _ap(alpha, 0, [[64, CF], [0, 2], [1, 64]])
            i_ag = nc.vector.tensor_tensor(
                out=ag.rearrange("p (f b s) -> p f b s", f=CF, b=2),
                in0=_ap(gp, goff, [[128, CF], [64, 2], [1, 64]]),
                in1=a_b, op=mult)
            add_dep_helper(i_ag.ins, i_d2.ins, False, "dve-order")
            # pairsum -> alpha' (prenorm); store to ahist when in output range
            if L <= tau < L + S:
                dst = ahist[:, (tau - L) * 256:(tau - L + 1) * 256]
            else:
                dst = nxt("aw", aw_bufs)[:]
            # split: f 0..PSF_F-1 on GPSIMD, rest on DVE
            i_pp = nc.gpsimd.tensor_tensor(
                out=_ap(dst, 0, [[64, PSF_F], [32, 2], [1, 32]]),
                in0=_ap(ag, 0, [[128, PSF_F], [64, 2], [2, 32]]),
                in1=_ap(ag, 1, [[128, PSF_F], [64, 2], [2, 32]]),
                op=add)
            state["last_pool_ins_f"] = i_pp
            state["last_pool_dst"] = dst
            if CF > PSF_F:
                nc.vector.tensor_tensor(
                    out=_ap(dst, 64 * PSF_F,
                            [[64, CF - PSF_F], [32, 2], [1, 32]]),
                    in0=_ap(ag, 128 * PSF_F,
                            [[128, CF - PSF_F], [64, 2], [2, 32]]),
                    in1=_ap(ag, 128 * PSF_F + 1,
                            [[128, CF - PSF_F], [64, 2], [2, 32]]),
                    op=add)
            alpha = dst
            if tau % NORM_EVERY == NORM_EVERY - 1:
                alpha = norm_apply(dst, norm_prep(env["alpha_p2"]))
            env["alpha_p2"] = env.get("alpha_p1")
            env["alpha_p1"] = dst
            env["alpha"] = alpha

        def bwd_step(t, tau, env):
            g = state["gstep"]
            state["gstep"] = g + 1
            gpb = g_pb[(t // 2) % 4][:]
            goff = (t % 2) * 512
            beta = env["beta"]
            bg = nxt("ag", ag_bufs)[:]
            i_d1 = nc.vector.tensor_copy(
                out=dve_scr[0:1, g % 16:g % 16 + 1], in_=beta[0:1, 0:1])
            mark_d1(i_d1)
            i_d2 = nc.vector.tensor_copy(
                out=dve_scr[0:1, 16 + g % 16:17 + g % 16],
                in_=beta[0:1, 64 * PSF_B:64 * PSF_B + 1])
            add_dep_helper(i_d2.ins, i_d1.ins, False, "dve-order")
            # layout (f, b, m, k): all operands unit-stride innermost (2x mode)
            b_g = _ap(beta, 0, [[64, CF], [32, 2], [0, 2], [1, 32]])
            g_in = _ap(gpb, goff, [[128, CF], [64, 2], [32, 2], [1, 32]])
            bg_out = _ap(bg, 0, [[128, CF], [64, 2], [32, 2], [1, 32]])
            i_bg = nc.vector.tensor_tensor(out=bg_out, in0=g_in, in1=b_g,
                                           op=mult)
            add_dep_helper(i_bg.ins, i_d2.ins, False, "dve-order")
            # beta for step tau goes to bhist[tau-1-L] (jm_k pairs with
            # beta_{k+1}, the beta bg uses at step k+L)
            kb = tau - 1 - L
            if 0 <= kb < S:
                dst = bhist[:, kb * 256:(kb + 1) * 256]
            else:
                dst = nxt("aw", aw_bufs)[:]
            # iterate (f, m, k): out idx 64f + 2k + m ; bg idx 128f + b64 + 32m + k
            i_pp = nc.gpsimd.tensor_tensor(
                out=_ap(dst, 0, [[64, PSF_B], [1, 2], [2, 32]]),
                in0=_ap(bg, 0, [[128, PSF_B], [32, 2], [1, 32]]),
                in1=_ap(bg, 64, [[128, PSF_B], [32, 2], [1, 32]]),
                op=add)
            state["last_pool_ins_b"] = i_pp
            state["last_pool_dst"] = dst
            nc.vector.tensor_tensor(
                out=_ap(dst, 64 * PSF_B,
                        [[64, CF - PSF_B], [1, 2], [2, 32]]),
                in0=_ap(bg, 128 * PSF_B,
                        [[128, CF - PSF_B], [32, 2], [1, 32]]),
                in1=_ap(bg, 128 * PSF_B + 64,
                        [[128, CF - PSF_B], [32, 2], [1, 32]]),
                op=add)
            beta = dst
            if tau % NORM_EVERY == NORM_PHASE_B:
                old = env.get("beta_p2") if t > 14 else dst
                beta = norm_apply(dst, norm_prep(old))
            env["beta_p2"] = env.get("beta_p1")
            env["beta_p1"] = dst
            env["beta"] = beta

        env = {}
        state["env"] = env
        env["alpha"] = nxt("aw", aw_bufs)[:]
        nc.vector.memset(env["alpha"], 1.0 / 64)
        env["beta"] = nxt("aw", aw_bufs)[:]
        nc.vector.memset(env["beta"], 1.0 / 64)
        # matmul emission leads each recursion by 2 steps so each pair-exp
        # completes before the first multiply that reads it
        ltf_sb = load_window(0)
        emit_mm_f(0, ltf_sb, True)
        emit_mm_f(1, ltf_sb, False)
        ltb_sb = load_window((TL - 1) // W)
        emit_mm_b(TL - 1, ltb_sb, True)
        emit_mm_b(TL - 2, ltb_sb, False)
        # Useful work ends at t = TLOOP-1 = S+L-1: fwd's last ahist write is
        # at t = L+S-1; bwd's last bhist write (kb=0) is at t = TLOOP-2.
        # Iterations beyond that only decay warm-down state nobody reads.
        TLOOP = S + L
        for t in range(TLOOP):
            tau_b = TL - 1 - t
            if t % W == W - 2 and t < TLOOP - 2:
                ltf_sb = load_window((t + 2) // W)
            if t < TLOOP - 2:
                emit_mm_f(t + 2, ltf_sb, t % W == W - 2)
            tau_e = tau_b - 2
            if t < TLOOP - 2:
                fresh_e = tau_e % W == W - 1
                if fresh_e:
                    ltb_sb = load_window(tau_e // W)
                emit_mm_b(tau_e, ltb_sb, fresh_e)
            fwd_step(t, env)
            if t <= TLOOP - 2:
                bwd_step(t, tau_b, env)

        # ---------------- epilogue: jm = ahist*bhist, half-sum tree --------
        # Split across engines: Pool (idle post-loop) takes jm+first tree
        # level of the last NPOOL blocks; DVE does the rest and all tails.
        # absorber: one explicit wait on the final GPSIMD pairsum covers all
        # Pool-side writes of ahist/bhist, so each jm keeps only its DVE wait
        NPOOL = 2
        NBLK = S // 16
        tr_pool_t = ctx.enter_context(nc.sbuf_tensor("trpool", [128, 2048], BF16))

        def tree_tail(cur, width, blk, eng_first=None):
            """DVE halving tree from `width` down to the fp32 jsum write."""
            first = True
            while width > 2:
                half = width // 2
                t_out = nxt("tr", tr_bufs)[:]
                i0 = _ap(cur, 0, [[width, 128], [1, half]])
                i1 = _ap(cur, half, [[width, 128], [1, half]])
                i_t = nc.vector.tensor_tensor(
                    out=_ap(t_out, 0, [[half, 128], [1, half]]),
                    in0=i0, in1=i1, op=add)
                if first and eng_first is not None:
                    add_dep_helper(i_t.ins, eng_first.ins, False, "dve-order")
                first = False
                cur = t_out
                width = half
            # final level: 2 -> 1, fp32 out into jsum (cols k*8 + f*2 + b)
            i0 = _ap(cur, 0, [[2, 128]])
            i1 = _ap(cur, 1, [[2, 128]])
            nc.vector.tensor_tensor(
                out=_ap(jsum, blk * 128, [[1, 128]]),
                in0=i0, in1=i1, op=add)

        # Pool handles jm+L1 of the last NPOOL blocks; both L1s write the
        # same tr_pool (a full L1 output is 2048 dense cols). The DVE tail of
        # the first Pool block is emitted between the two Pool blocks so
        # tile's WAR sem makes L1(second) wait for its read.
        def pool_block(blk, prev_pool):
            base = blk * 16 * 256
            jm = jm_bufs[1][:]
            i_pjm = nc.gpsimd.tensor_tensor(
                out=jm, in0=ahist[:, base:base + 4096],
                in1=bhist[:, base:base + 4096], op=mult)
            if prev_pool is None:
                add_dep_helper(i_pjm.ins, state["last_pool_ins_f"].ins, False,
                               "pool-order")
                add_dep_helper(i_pjm.ins, state["last_pool_ins_b"].ins, False,
                               "pool-order")
            else:
                add_dep_helper(i_pjm.ins, prev_pool.ins, False, "pool-order")
            i_l1 = nc.gpsimd.tensor_tensor(
                out=_ap(tr_pool_t[:], 0, [[16, 128], [1, 16]]),
                in0=_ap(jm, 0, [[32, 128], [1, 16]]),
                in1=_ap(jm, 16, [[32, 128], [1, 16]]),
                op=add)
            return i_l1

        def pool_tail(blk, j, prev_dve):
            # absorber: i_pt carries the Pool wait (that block's L1 write) so
            # the first tree level keeps only its DVE-self wait
            i_pt = nc.vector.tensor_copy(
                out=dve_scr[0:1, 35 + j:36 + j], in_=tr_pool_t[0:1, 0:1])
            add_dep_helper(i_pt.ins, prev_dve.ins, False, "dve-order")
            cur = _ap(tr_pool_t[:], 0, [[1, 2048]])
            tree_tail(cur, 16, blk, eng_first=i_pt)
            return i_pt

        # ---------------- epilogue: llr = ln(j0 / j1), split in k-halves ---
        # ratio first: j0/j1 = exp(llr) stays in the ACT Ln table's valid
        # input range, while raw jsum values (prenorm products) can reach
        # e^70 and fall off the table. Half A (k 0..63, jm blocks 0-3) is
        # emitted mid-epilogue so its Ln + store DMA overlap the remaining
        # blocks; half B finishes after the Pool-block tails.
        rat_t = ctx.enter_context(nc.sbuf_tensor("ratbuf", [128, 512], F32))
        rat = rat_t[:]
        llr_t2 = ctx.enter_context(nc.sbuf_tensor("llrsb", [128, 512], F32))
        llr_sb = llr_t2

        def llr_half(h):
            rcp = llr_sb[:]  # scratch for 1/j1 before Ln overwrites it
            in0 = _ap(jsum, h * 512, [[2, CF], [8, S // 2]])
            in1 = _ap(jsum, h * 512 + 1, [[2, CF], [8, S // 2]])
            rcp_h = _ap(rcp, h * 64, [[128, CF], [1, S // 2]])
            rat_h = _ap(rat, h * 64, [[128, CF], [1, S // 2]])
            nc.vector.reciprocal(out=rcp_h, in_=in1)
            nc.vector.tensor_tensor(out=rat_h, in0=in0, in1=rcp_h, op=mult)
            nc.scalar.activation(out=_ap(llr_sb[:], h * 64,
                                         [[128, CF], [1, S // 2]]),
                                 in_=rat_h,
                                 func=mybir.ActivationFunctionType.Ln)
            if h == 1:
                # single fused store (two DMAs would land on two HW queues
                # and the exit drain can carry only one sem wait)
                src_ap = llr_sb[:].rearrange("p (f k) -> p f k", f=4)
                dst_ap = bass.AP(tensor=out_d[:].tensor, offset=0,
                                 ap=[[2048, 32], [512, 4], [128, 4], [1, 128]])
                nc.sync.dma_start(out=dst_ap, in_=src_ap)

        i_l1a = pool_block(NBLK - 2, None)

        # blocks of 16 k-steps: jm [128, 16*256]; cols k(16) f(4) b(2) m(32)
        # bhist[0] (bwd t=TLOOP-2) is the last-scheduled Pool history write;
        # Pool is in-order so waiting on it covers the fwd one too
        i_ep = nc.vector.tensor_copy(out=dve_scr[0:1, 33:34],
                                     in_=bhist[0:1, 0:1])
        prev_ep = i_ep
        for blk in range(NBLK - NPOOL):
            base = blk * 16 * 256
            jm = jm_bufs[0][:]
            i_jm = nc.vector.tensor_tensor(
                out=jm, in0=ahist[:, base:base + 4096],
                in1=bhist[:, base:base + 4096], op=mult)
            add_dep_helper(i_jm.ins, prev_ep.ins, False, "dve-order")
            prev_ep = i_jm
            tree_tail(jm, 32, blk)
            if blk == 3:
                # tail of the first Pool block, early enough that the second
                # Pool block's L1 never stalls on its tr_pool read
                prev_ep = pool_tail(NBLK - 2, 0, prev_ep)
                llr_half(0)
        i_l1b = pool_block(NBLK - 1, i_l1a)
        pool_tail(NBLK - 1, 1, prev_ep)
        llr_half(1)

        if dbg:
            nc.sync.dma_start(out=dbg_jsum[:], in_=jsum)
            nc.sync.dma_start(out=dbg_ah[:, 0:256], in_=ahist[:, 0:256])
            nc.sync.dma_start(out=dbg_ah[:, 256:512], in_=ahist[:, 64*256:64*256+256])
            nc.sync.dma_start(out=dbg_bh[:, 0:256], in_=bhist[:, 0:256])
            nc.sync.dma_start(out=dbg_bh[:, 256:512], in_=bhist[:, 64*256:64*256+256])
            nc.sync.dma_start(out=dbg_jm[:], in_=jm_bufs[0][:][:, 0:512])

    return nc


_ENG_SELF = {"PE": "PE_", "DVE": "DVE_", "Activation": "Activation_",
             "Pool": "Pool_", "SP": "SP_"}


def _prune_waits(nc):
    """Drop sem waits already implied, so each instruction carries <=1.

    HW structs accept one sync wait per instruction. Tile emits waits that
    are provably satisfied at issue. Vector-clock rules:
      - cross-engine sems: knowledge from transitive joins of kept waits
      - self sems (same engine): only monotone vs explicitly-waited values
        (ACT/DVE completion is not implied by issue order); PE and DMA
        queues complete in order, so own-increment knowledge counts there.
    """
    know = {}        # proc -> {sem_id: known completed value}
    waited_max = {}  # proc -> {sem_id: max explicitly waited}
    sem_total = {}   # sem_id -> running total
    hist = {}        # sem_id -> [(total_after, snapshot)]
    out_dma_sems = set()
    bad = []
    for b in nc.m.functions[0].blocks:
        for i in b.instructions:
            si = i.sync_info
            op = str(getattr(i, "opcode", type(i).__name__))
            if si is None:
                continue
            upds = [u for u in (si.on_update or [])
                    if u.sync_type == "semaphore"
                    and u.update_mode in ("sem-inc", "sem-add-imm")]
            if "DMACopy" in op and upds:
                proc = str(upds[0].ant_name)
                outs = getattr(i, "outs", None) or []
                if outs and "llr_out" in str(getattr(outs[0], "memref", "")):
                    out_dma_sems.add(upds[0].id)
            else:
                proc = getattr(i.engine, "value", str(i.engine))
            k = know.setdefault(proc, {})
            wm = waited_max.setdefault(proc, {})
            in_order = (proc == "PE" or proc == "Pool"
                        or proc.startswith("DMAHW"))
            if "Drain" in op and si.on_wait and len(si.on_wait) > 1:
                keep_d = [w for w in si.on_wait if w.id in out_dma_sems]
                # several output DMAs on one queue: keep only the max-value
                # wait per sem (the queue completes in order)
                best = {}
                for w in keep_d:
                    b = best.get(w.id)
                    if b is None or (w.wait_value or 0) > (b.wait_value or 0):
                        best[w.id] = w
                si.on_wait = list(best.values())
                continue
            skip = ("Drain" in op) or ("EventSem" in op)
            ow = list(si.on_wait or [])
            if ow and not skip:
                keep = []
                for w in ow:
                    if (w.sync_type != "semaphore"
                            or w.wait_mode != "sem-ge-imm"
                            or w.wait_value is None
                            or str(w.ant_name).startswith("barrier")):
                        keep.append(w)
                        continue
                    v = w.wait_value
                    nm = str(w.ant_name)
                    is_self = nm == proc or nm.startswith(proc + "_")
                    if is_self:
                        implied = (wm.get(w.id, -1) >= v
                                   or (in_order and k.get(w.id, 0) >= v))
                    else:
                        implied = (k.get(w.id, 0) >= v
                                   or wm.get(w.id, -1) >= v)
                    if implied:
                        continue
                    keep.append(w)
                    wm[w.id] = max(wm.get(w.id, -1), v)
                    for tot, snap in hist.get(w.id, ()):
                        if tot >= v:
                            for s2, v2 in snap.items():
                                if k.get(s2, 0) < v2:
                                    k[s2] = v2
                            break
                    if k.get(w.id, 0) < v:
                        k[w.id] = v
                if len(keep) != len(ow):
                    si.on_wait = keep
                    ow = keep
                if len(ow) > 1:
                    bad.append((i.name, op,
                                [(x.ant_name, x.wait_value) for x in ow]))
            for u in upds:
                tot = sem_total.get(u.id, 0) + (u.update_value or 0)
                sem_total[u.id] = tot
                k[u.id] = tot
                hist.setdefault(u.id, []).append((tot, dict(k)))
    if bad:
        raise RuntimeError(f"{len(bad)} insts still multi-wait: {bad[:8]}")
    return nc


def _get_nc():
    if "nc" not in _NC_CACHE:
        _NC_CACHE["nc"] = _prune_waits(build_nc())
    return _NC_CACHE["nc"]


# ---------------- host-side layout ----------------
def _prep_core(llr_ch_c, llr_a_c):
    """llr_ch_c [32, 4096], llr_a_c [32, 2048] -> llr_t [12, TL*128] bf16."""
    lc = np.zeros((B_CORE, T + 2 * L, 2), np.float32)
    lc[:, L:L + T] = llr_ch_c.reshape(B_CORE, T, 2)
    la = np.full((B_CORE, T + 2 * L), PAD_A, np.float32)
    la[:, L:L + T] = llr_a_c
    # windows [B, C, TL, comp]
    idx = (np.arange(C)[:, None] * S + np.arange(TL)[None, :])  # [C, TL]
    w = np.stack([la[:, idx], lc[:, idx, 0], lc[:, idx, 1]], -1)  # [B, C, TL, 3]
    # chunk c = g*4+f ; row = cw*4+g ; llr_t[f*3+comp, tau*128+row]
    w = w.reshape(B_CORE, 4, 4, TL, 3)            # [cw, g, f, tau, comp]
    w = w.transpose(2, 4, 3, 0, 1)                # [f, comp, tau, cw, g]
    return np.ascontiguousarray(
        w.reshape(12, TL * 128)).astype(ml_dtypes.bfloat16)


def _run(llr_ch, llr_a, trace=False):
    nc = _get_nc()
    in_maps = []
    for core in range(N_CORES):
        sl = slice(core * B_CORE, (core + 1) * B_CORE)
        in_maps.append({
            "llr_t": _prep_core(np.asarray(llr_ch[sl], np.float32),
                                np.asarray(llr_a[sl], np.float32)),
            "sign": SIGN_BD,
        })
    res = run_bass_kernel_spmd(nc, in_maps, core_ids=list(range(N_CORES)),
                               trace=trace)
    out = np.concatenate([r["llr_out"] for r in res.results], 0)
    return out.astype(np.float32), res


def kernel(llr_ch, llr_a):
    out, _ = _run(llr_ch, llr_a, trace=False)
    return out



# revision 54
# speedup vs baseline: 1.0029x; 1.0009x over previous
"""BCJR decoder (rate-1/2 conv code, 64 states) on 8 Trainium2 cores.

Strategy
--------
Data-parallel over batch: 32 codewords per core. Within a core, each
codeword's T=2048 trellis steps are split into C=16 chunks of 128 steps,
decoded in parallel with L=12 warm-up steps on each side (windowed BCJR).
The time axis is padded with llr_a=+8 "pilot" steps which deterministically
collapse the state to 0, making chunk 0 / chunk 15 boundary conditions exact.

Layout: 128 SBUF partitions = 32 codewords x 4 chunk-groups; 4 more chunks
("f groups") along the free dimension. The interleaved fwd+bwd loop runs
only S+L = 140 iterations: fwd's last ahist write is at t = S+L-1 and bwd's
last bhist write at t = S+L-2, so the remaining warm-down steps are skipped.

Per step: PE matmul (bf16 sign-table x llr triple) builds branch-metric
exponents E in PSUM (two steps batched per buffer); ScalarE does ONE
exp(0.5 E) per 2-step pair for each chain (the per-instruction SBUF-access
overhead on ACT is ~230 ns, so pairing halves it); VectorE does the
alpha/beta gather-mults; the pairwise adds are split GPSIMD/DVE per chain
(PSF_F/PSF_B f-chunks on Pool, rest on DVE — tuned against the cost model).
Normalization runs every NORM_EVERY=32 steps with a stale (2-step-old)
denominator so the reduce+reciprocal sit off the serial path; the bwd
phase is offset (NORM_PHASE_B) so the prenorm alpha/beta peaks never
coincide in the jm product (bf16 overflow guard — NE=48 aligned NaNs).

Epilogue: jm = ahist*bhist and the per-(f,b) m-sum tree. The last NPOOL=2
blocks' jm+first tree level run on GPSIMD (pinned after the loop's final
pairsums — Pool is in-order, so an early big jm would block the recursion);
DVE does the rest, with the LLR transform of the first k-half emitted
mid-epilogue so its Ln overlaps the remaining tree work.
"""

import os
from contextlib import ExitStack

import numpy as np
import ml_dtypes

import concourse.bass as bass
import concourse.mybir as mybir
from concourse import tile as tile_mod
from concourse.tile_rust import add_dep_helper
from concourse.bass_utils import run_bass_kernel_spmd

# ---------------- problem constants (hardcoded) ----------------
B_FULL, N_FULL = 256, 4096
T = N_FULL // 2            # 2048 trellis steps
N_CORES = 8
B_CORE = B_FULL // N_CORES  # 32 codewords per core
C = 16                     # time chunks per codeword
S = T // C                 # 128 steps per chunk
L = 12                     # warmup steps each side
TL = S + 2 * L             # 160 local steps
CF = 4                     # chunks in free dim (C = 4 partition-groups * CF)
PAD_A = 8.0                # llr_a pad value (forces state collapse)
NORM_EVERY = 32
NORM_PHASE_B = 16        # bwd normalize phase: offset so prenorm peaks of
                         # alpha/beta never coincide in the jm product
PSF_F = 2                  # fwd pairsum: f-chunks 0..PSF_F-1 on GPSIMD, rest DVE
PSF_B = 2                  # bwd pairsum: f-chunks 0..PSF_B-1 on GPSIMD, rest DVE

F32 = mybir.dt.float32
BF16 = mybir.dt.bfloat16


def _sign_table():
    """[3, 128] rows (la, l0, l1) x cols (b, s): E[s,b] = sum_c sign[c,(b,s)] * llr_c."""
    gen = ("1111001", "1011011")
    mu = 6
    g = np.array([[int(c) for c in p] for p in gen])
    opf = np.zeros((64, 2), np.int32)
    for s in range(64):
        rbits = [(s >> (mu - 1 - j)) & 1 for j in range(mu)]
        for b in range(2):
            w = np.array([b] + rbits)
            obits = (g @ w) % 2
            opf[s, b] = obits[0] * 2 + obits[1]
    ops = (1.0 - 2.0 * np.array([[(o >> (1 - j)) & 1 for j in range(2)]
                                 for o in range(4)])).astype(np.float32)
    sa = np.concatenate([np.ones(64), -np.ones(64)])
    s0 = np.concatenate([ops[opf[:, 0], 0], ops[opf[:, 1], 0]])
    s1 = np.concatenate([ops[opf[:, 0], 1], ops[opf[:, 1], 1]])
    return np.stack([sa, s0, s1]).astype(np.float32)  # [3, 128]


SIGN_NP = _sign_table()
# block-diag [12, 512]: rows (f*3+c), cols (f', (b,s))
SIGN_BD = np.zeros((12, 512), np.float32)
for _f in range(4):
    SIGN_BD[_f * 3:_f * 3 + 3, _f * 128:(_f + 1) * 128] = SIGN_NP
SIGN_BD = SIGN_BD.astype(ml_dtypes.bfloat16)

# ---------------- bass program ----------------
_NC_CACHE = {}

W = 19                 # llr_t streaming window (steps)
NW = TL // W           # 8 windows


def _ap(a, offset_extra, dims):
    """Custom AP over the same tensor as `a` (partition dim kept)."""
    return bass.AP(tensor=a.tensor, offset=a.offset + offset_extra,
                   ap=[list(a.ap[0])] + [list(d) for d in dims])


def build_nc():
    nc = bass.Bass()
    llr_t_d = nc.declare_dram_parameter("llr_t", [12, TL * 128], BF16, isOutput=False)
    sign_d = nc.declare_dram_parameter("sign", [12, 512], BF16, isOutput=False)
    out_d = nc.declare_dram_parameter("llr_out", [B_CORE, T], F32, isOutput=True)
    dbg = os.environ.get("KDBG", "0") == "1"
    if dbg:
        dbg_jsum = nc.declare_dram_parameter("dbg_jsum", [128, S * 8], F32, isOutput=True)
        dbg_ah = nc.declare_dram_parameter("dbg_ah", [128, 512], BF16, isOutput=True)
        dbg_bh = nc.declare_dram_parameter("dbg_bh", [128, 512], BF16, isOutput=True)
        dbg_jm = nc.declare_dram_parameter("dbg_jm", [128, 512], BF16, isOutput=True)

    mult = mybir.AluOpType.mult
    add = mybir.AluOpType.add

    with tile_mod.TileContext(nc) as tc, ExitStack() as ctx:
        # static ring buffers (pool alloc/release deps would exceed the
        # 1-sync-wait-per-instruction hardware limit)
        def ring(nm, n, shape, dt=F32):
            return [ctx.enter_context(nc.sbuf_tensor(f"{nm}{i}", shape, dt))
                    for i in range(n)]

        e_pb = [ctx.enter_context(nc.psum_tensor(f"epb{_i}", [128, 1024], F32))
                for _i in range(2)]
        e_pf = [ctx.enter_context(nc.psum_tensor(f"epf{_i}", [128, 1024], F32))
                for _i in range(2)]
        g_pb = ring("gpb", 4, [128, 1024], BF16)
        g_pf = ring("gpf", 3, [128, 1024], BF16)
        ag_bufs = ring("agbuf", 4, [128, 512], BF16)
        aw_bufs = ring("awbuf", 6, [128, 256], BF16)
        nrm_bufs = ring("nrmbuf", 2, [128, 2 * CF])
        lt_bufs = ring("ltbuf", 4, [12, W * 128], BF16)
        jm_bufs = ring("jmblk", 2, [128, 16 * 256], BF16)
        tr_bufs = ring("trbuf", 2, [128, 2048], BF16)

        dve_scr = ctx.enter_context(nc.sbuf_tensor("dvescr", [1, 37], F32))
        act_scr = ctx.enter_context(nc.sbuf_tensor("actscr", [1, 8], F32))
        act_scr2 = ctx.enter_context(nc.sbuf_tensor("actscr2", [1, 8], F32))
        # fwd pair-exp absorbers: unique column per pair (a reused slot's
        # WAW wait gets spilled by tile onto the next engine instruction)
        act_scr3 = ctx.enter_context(nc.sbuf_tensor("actscr3", [1, 180], F32))
        act_scr4 = ctx.enter_context(nc.sbuf_tensor("actscr4", [1, 180], F32))
        sign_t = ctx.enter_context(nc.sbuf_tensor("sign_sb", [12, 512], BF16))
        sign_sb = sign_t[:]
        nc.gpsimd.dma_start(out=sign_sb, in_=sign_d[:])

        state = {"prev_g": None, "gstep": 0, "d1_bufs": [None] * 16,
                 "nb": 0, "nf": 0, "nbw": 0, "env": None}
        ahist_t = ctx.enter_context(nc.sbuf_tensor("ahist", [128, S * 256], BF16))
        ahist = ahist_t[:]
        bhist_t = ctx.enter_context(nc.sbuf_tensor("bhist", [128, S * 256], BF16))
        bhist = bhist_t[:]
        jsum_t = ctx.enter_context(nc.sbuf_tensor("jsum", [128, S * 8], F32))
        jsum = jsum_t[:]
        _counters = {"g": 0, "ag": 0, "aw": 0, "nrm": 0, "lt": 0, "jm": 0, "tr": 0}

        def nxt(nm, bufs):
            i = _counters[nm]
            _counters[nm] = i + 1
            return bufs[i % len(bufs)]

        def emit_mm_b(tau, lt_sb, fresh_dma):
            """Bwd: matmul for step tau into half of a PSUM pair; one
            deinterleaved exp per pair (runs 2 steps ahead of the recursion).

            PE Matmult (LW struct) supports only ONE sync wait, so 1-element
            dummy matmuls absorb the PSUM-WAR and window-DMA waits first.
            """
            n = state["nbw"]
            state["nbw"] = n + 1
            pidx, half = divmod(n, 2)
            e_pair = e_pb[pidx % 2]
            col = (tau % W) * 128
            off = half * 512
            nc.tensor.matmul(out=e_pair[0:1, off:off + 1],
                             lhsT=sign_t[0:1, 0:1],
                             rhs=sign_t[0:1, 0:1], start=True, stop=True)
            if fresh_dma:
                nc.tensor.matmul(out=e_pair[0:1, off:off + 1],
                                 lhsT=lt_sb[0:1, col:col + 1],
                                 rhs=sign_t[0:1, 0:1], start=True, stop=True)
            nc.tensor.matmul(
                out=e_pair[:, off:off + 512], lhsT=lt_sb[:, col:col + 128],
                rhs=sign_sb, start=True, stop=True)
            if half == 1:
                gpb = g_pb[pidx % 4][:]
                # absorber chain: anchor (ACT self-progress via previous bwd
                # pair), c2a (DVE progress covering the g_pb WAR), then exp
                # carries only the PE wait. See emit_mm_f.
                a_src = g_pb[(pidx - 1) % 4][:] if pidx >= 1 else sign_sb
                i_anc = nc.scalar.copy(out=act_scr4[0:1, pidx:pidx + 1],
                                       in_=a_src[0:1, 0:1])
                g = state["gstep"]
                if g >= 2 and state["d1_bufs"][(g - 2) % 16] is not None:
                    dsrc = state["d1_bufs"][(g - 2) % 16]
                else:
                    dsrc = sign_sb[0:1, 0:1]
                i_c2a = nc.scalar.copy(out=act_scr4[0:1, 90 + pidx:91 + pidx],
                                       in_=dsrc)
                add_dep_helper(i_c2a.ins, i_anc.ins, False, "act-order")
                # both halves deinterleaved: within half h (cols h*512..),
                # col (f, b, k, i) = 128f + 64b + k + 32i  <-  E (f, b, s=2k+i)
                i_exp = nc.scalar.activation(
                    out=_ap(gpb, 0, [[128, 8], [64, 2], [1, 32], [32, 2]]),
                    in_=_ap(e_pair[:], 0, [[128, 8], [64, 2], [2, 32], [1, 2]]),
                    func=mybir.ActivationFunctionType.Exp, scale=0.5)
                add_dep_helper(i_exp.ins, i_c2a.ins, False, "act-order")

        def emit_mm_f(tau, lt_sb, fresh_dma):
            """Fwd: matmul for step tau into half of a PSUM pair; one exp
            per pair (runs 2 steps ahead of the recursion)."""
            n = state["nf"]
            state["nf"] = n + 1
            pidx, half = divmod(n, 2)
            e_pair = e_pf[pidx % 2]
            col = (tau % W) * 128
            off = half * 512
            nc.tensor.matmul(out=e_pair[0:1, off:off + 1],
                             lhsT=sign_t[0:1, 0:1],
                             rhs=sign_t[0:1, 0:1], start=True, stop=True)
            if fresh_dma:
                nc.tensor.matmul(out=e_pair[0:1, off:off + 1],
                                 lhsT=lt_sb[0:1, col:col + 1],
                                 rhs=sign_t[0:1, 0:1], start=True, stop=True)
            nc.tensor.matmul(
                out=e_pair[:, off:off + 512], lhsT=lt_sb[:, col:col + 128],
                rhs=sign_sb, start=True, stop=True)
            if half == 1:
                gp = g_pf[pidx % 3][:]
                # anchor: ACT-self wait on the PREVIOUS fwd pair's exp (fresh
                # waited_max so stale WAW/WAR self-waits prune; own slot would
                # create an unprunable WAR of the exp on its own anchor)
                a_src = g_pf[(pidx - 1) % 3][:] if pidx >= 1 else sign_sb
                i_anc = nc.scalar.copy(out=act_scr3[0:1, pidx:pidx + 1],
                                       in_=a_src[0:1, 0:1])
                g = state["gstep"]
                if g >= 2 and state["d1_bufs"][(g - 2) % 16] is not None:
                    dsrc = state["d1_bufs"][(g - 2) % 16]
                else:
                    dsrc = sign_sb[0:1, 0:1]
                i_c2a = nc.scalar.copy(out=act_scr3[0:1, 90 + pidx:91 + pidx],
                                       in_=dsrc)
                add_dep_helper(i_c2a.ins, i_anc.ins, False, "act-order")
                i_exp = nc.scalar.activation(
                    out=_ap(gp, 0, [[128, 8], [64, 2], [1, 64]]),
                    in_=_ap(e_pair[:], 0, [[128, 8], [64, 2], [1, 64]]),
                    func=mybir.ActivationFunctionType.Exp, scale=0.5)
                add_dep_helper(i_exp.ins, i_c2a.ins, False, "act-order")

        def mark_d1(i_d1_src):
            """Record the dve_scr slot source for step g (for c2a 4 steps on)."""
            g = state["gstep"] - 1
            state["d1_bufs"][g % 16] = dve_scr[0:1, g % 16:g % 16 + 1]

        def load_window(w):
            lt_sb = nxt("lt", lt_bufs)
            nc.sync.dma_start(out=lt_sb[:],
                              in_=llr_t_d[:, w * W * 128:(w + 1) * W * 128])
            return lt_sb

        def norm_prep(cur):
            """Emit reduce+recip on the CURRENT state, two iterations before
            the normalize that uses it. Any positive per-(row,f) scale is
            valid (it only guards bf16 range and cancels in the LLR ratio),
            so the denominator can be two steps stale -- this keeps the
            reduce+recip and their pairsum wait off the apply step's path."""
            nb = nxt("nrm", nrm_bufs)
            asum = nb[:, 0:CF]
            # absorber: cur has a GPSIMD writer and a DVE writer;
            # i_n carries the Pool wait so the reduce keeps only its DVE wait
            i_n = nc.vector.tensor_copy(out=dve_scr[0:1, 32:33],
                                        in_=cur[0:1, 0:1])
            i_rd = nc.vector.tensor_reduce(
                out=asum, in_=cur.rearrange("p (f s) -> p f s", f=CF),
                axis=mybir.AxisListType.X, op=add)
            add_dep_helper(i_rd.ins, i_n.ins, False, "dve-order")
            rz = nb[:, CF:2 * CF]
            nc.vector.reciprocal(out=rz, in_=asum)
            return rz

        def norm_apply(cur, rz):
            anorm = nxt("aw", aw_bufs)[:]
            rz_b = _ap(rz, 0, [[1, CF], [0, 64]])
            # absorber: cur is fresh (Pool + DVE writers); i_n2 carries the
            # Pool wait so the apply-mult keeps only its DVE wait
            i_n2 = nc.vector.tensor_copy(out=dve_scr[0:1, 34:35],
                                         in_=cur[0:1, 0:1])
            i_ap2 = nc.vector.tensor_tensor(
                out=anorm.rearrange("p (f s) -> p f s", f=CF),
                in0=cur.rearrange("p (f s) -> p f s", f=CF),
                in1=rz_b, op=mult)
            add_dep_helper(i_ap2.ins, i_n2.ins, False, "dve-order")
            return anorm

        # ------------- interleaved forward + backward -------------
        # Two independent recursion chains share each engine; while one
        # chain's DVE self-semaphore propagates, the other chain's ops
        # execute, so the period is engine-busy-bound, not latency-bound.
        def fwd_step(tau, env):
            g = state["gstep"]
            state["gstep"] = g + 1
            gp = g_pf[(tau // 2) % 3][:]
            goff = (tau % 2) * 512
            alpha = env["alpha"]
            ag = nxt("ag", ag_bufs)[:]
            # DVE absorbers: i_d1 carries the Pool wait (alpha f0-2 part),
            # i_d2 the DVE self-wait (alpha f3 part); mult keeps only ACT
            i_d1 = nc.vector.tensor_copy(
                out=dve_scr[0:1, g % 16:g % 16 + 1], in_=alpha[0:1, 0:1])
            mark_d1(i_d1)
            i_d2 = nc.vector.tensor_copy(
                out=dve_scr[0:1, 16 + g % 16:17 + g % 16],
                in_=alpha[0:1, 64 * PSF_F:64 * PSF_F + 1])
            add_dep_helper(i_d2.ins, i_d1.ins, False, "dve-order")
            a_b = _ap(alpha, 0, [[64, CF], [0, 2], [1, 64]])
            i_ag = nc.vector.tensor_tensor(
                out=ag.rearrange("p (f b s) -> p f b s", f=CF, b=2),
                in0=_ap(gp, goff, [[128, CF], [64, 2], [1, 64]]),
                in1=a_b, op=mult)
            add_dep_helper(i_ag.ins, i_d2.ins, False, "dve-order")
            # pairsum -> alpha' (prenorm); store to ahist when in output range
            if L <= tau < L + S:
                dst = ahist[:, (tau - L) * 256:(tau - L + 1) * 256]
            else:
                dst = nxt("aw", aw_bufs)[:]
            # split: f 0..PSF_F-1 on GPSIMD, rest on DVE
            i_pp = nc.gpsimd.tensor_tensor(
                out=_ap(dst, 0, [[64, PSF_F], [32, 2], [1, 32]]),
                in0=_ap(ag, 0, [[128, PSF_F], [64, 2], [2, 32]]),
                in1=_ap(ag, 1, [[128, PSF_F], [64, 2], [2, 32]]),
                op=add)
            state["last_pool_ins_f"] = i_pp
            state["last_pool_dst"] = dst
            if CF > PSF_F:
                nc.vector.tensor_tensor(
                    out=_ap(dst, 64 * PSF_F,
                            [[64, CF - PSF_F], [32, 2], [1, 32]]),
                    in0=_ap(ag, 128 * PSF_F,
                            [[128, CF - PSF_F], [64, 2], [2, 32]]),
                    in1=_ap(ag, 128 * PSF_F + 1,
                            [[128, CF - PSF_F], [64, 2], [2, 32]]),
                    op=add)
            alpha = dst
            if tau % NORM_EVERY == NORM_EVERY - 1:
                alpha = norm_apply(dst, norm_prep(env["alpha_p2"]))
            env["alpha_p2"] = env.get("alpha_p1")
            env["alpha_p1"] = dst
            env["alpha"] = alpha

        def bwd_step(t, tau, env):
            g = state["gstep"]
            state["gstep"] = g + 1
            gpb = g_pb[(t // 2) % 4][:]
            goff = (t % 2) * 512
            beta = env["beta"]
            bg = nxt("ag", ag_bufs)[:]
            i_d1 = nc.vector.tensor_copy(
                out=dve_scr[0:1, g % 16:g % 16 + 1], in_=beta[0:1, 0:1])
            mark_d1(i_d1)
            i_d2 = nc.vector.tensor_copy(
                out=dve_scr[0:1, 16 + g % 16:17 + g % 16],
                in_=beta[0:1, 64 * PSF_B:64 * PSF_B + 1])
            add_dep_helper(i_d2.ins, i_d1.ins, False, "dve-order")
            # layout (f, b, m, k): all operands unit-stride innermost (2x mode)
            b_g = _ap(beta, 0, [[64, CF], [32, 2], [0, 2], [1, 32]])
            g_in = _ap(gpb, goff, [[128, CF], [64, 2], [32, 2], [1, 32]])
            bg_out = _ap(bg, 0, [[128, CF], [64, 2], [32, 2], [1, 32]])
            i_bg = nc.vector.tensor_tensor(out=bg_out, in0=g_in, in1=b_g,
                                           op=mult)
            add_dep_helper(i_bg.ins, i_d2.ins, False, "dve-order")
            # beta for step tau goes to bhist[tau-1-L] (jm_k pairs with
            # beta_{k+1}, the beta bg uses at step k+L)
            kb = tau - 1 - L
            if 0 <= kb < S:
                dst = bhist[:, kb * 256:(kb + 1) * 256]
            else:
                dst = nxt("aw", aw_bufs)[:]
            # iterate (f, m, k): out idx 64f + 2k + m ; bg idx 128f + b64 + 32m + k
            i_pp = nc.gpsimd.tensor_tensor(
                out=_ap(dst, 0, [[64, PSF_B], [1, 2], [2, 32]]),
                in0=_ap(bg, 0, [[128, PSF_B], [32, 2], [1, 32]]),
                in1=_ap(bg, 64, [[128, PSF_B], [32, 2], [1, 32]]),
                op=add)
            state["last_pool_ins_b"] = i_pp
            state["last_pool_dst"] = dst
            nc.vector.tensor_tensor(
                out=_ap(dst, 64 * PSF_B,
                        [[64, CF - PSF_B], [1, 2], [2, 32]]),
                in0=_ap(bg, 128 * PSF_B,
                        [[128, CF - PSF_B], [32, 2], [1, 32]]),
                in1=_ap(bg, 128 * PSF_B + 64,
                        [[128, CF - PSF_B], [32, 2], [1, 32]]),
                op=add)
            beta = dst
            if tau % NORM_EVERY == NORM_PHASE_B:
                old = env.get("beta_p2") if t > 14 else dst
                beta = norm_apply(dst, norm_prep(old))
            env["beta_p2"] = env.get("beta_p1")
            env["beta_p1"] = dst
            env["beta"] = beta

        env = {}
        state["env"] = env
        env["alpha"] = nxt("aw", aw_bufs)[:]
        nc.vector.memset(env["alpha"], 1.0 / 64)
        env["beta"] = nxt("aw", aw_bufs)[:]
        nc.vector.memset(env["beta"], 1.0 / 64)
        # matmul emission leads each recursion by 2 steps so each pair-exp
        # completes before the first multiply that reads it
        ltf_sb = load_window(0)
        emit_mm_f(0, ltf_sb, True)
        emit_mm_f(1, ltf_sb, False)
        ltb_sb = load_window((TL - 1) // W)
        emit_mm_b(TL - 1, ltb_sb, True)
        emit_mm_b(TL - 2, ltb_sb, False)
        # Useful work ends at t = TLOOP-1 = S+L-1: fwd's last ahist write is
        # at t = L+S-1; bwd's last bhist write (kb=0) is at t = TLOOP-2.
        # Iterations beyond that only decay warm-down state nobody reads.
        TLOOP = S + L
        for t in range(TLOOP):
            tau_b = TL - 1 - t
            if t % W == W - 2 and t < TLOOP - 2:
                ltf_sb = load_window((t + 2) // W)
            if t < TLOOP - 2:
                emit_mm_f(t + 2, ltf_sb, t % W == W - 2)
            tau_e = tau_b - 2
            if t < TLOOP - 2:
                fresh_e = tau_e % W == W - 1
                if fresh_e:
                    ltb_sb = load_window(tau_e // W)
                emit_mm_b(tau_e, ltb_sb, fresh_e)
            fwd_step(t, env)
            if t <= TLOOP - 2:
                bwd_step(t, tau_b, env)

        # ---------------- epilogue: jm = ahist*bhist, half-sum tree --------
        # Split across engines: Pool (idle post-loop) takes jm+first tree
        # level of the last NPOOL blocks; DVE does the rest and all tails.
        # absorber: one explicit wait on the final GPSIMD pairsum covers all
        # Pool-side writes of ahist/bhist, so each jm keeps only its DVE wait
        NPOOL = 2
        NBLK = S // 16
        tr_pool_t = ctx.enter_context(nc.sbuf_tensor("trpool", [128, 2048], BF16))

        def tree_tail(cur, width, blk, eng_first=None):
            """DVE halving tree from `width` down to the fp32 jsum write."""
            first = True
            while width > 2:
                half = width // 2
                t_out = nxt("tr", tr_bufs)[:]
                i0 = _ap(cur, 0, [[width, 128], [1, half]])
                i1 = _ap(cur, half, [[width, 128], [1, half]])
                i_t = nc.vector.tensor_tensor(
                    out=_ap(t_out, 0, [[half, 128], [1, half]]),
                    in0=i0, in1=i1, op=add)
                if first and eng_first is not None:
                    add_dep_helper(i_t.ins, eng_first.ins, False, "dve-order")
                first = False
                cur = t_out
                width = half
            # final level: 2 -> 1, fp32 out into jsum (cols k*8 + f*2 + b)
            i0 = _ap(cur, 0, [[2, 128]])
            i1 = _ap(cur, 1, [[2, 128]])
            nc.vector.tensor_tensor(
                out=_ap(jsum, blk * 128, [[1, 128]]),
                in0=i0, in1=i1, op=add)

        # Pool handles jm+L1 of the last NPOOL blocks; both L1s write the
        # same tr_pool (a full L1 output is 2048 dense cols). The DVE tail of
        # the first Pool block is emitted between the two Pool blocks so
        # tile's WAR sem makes L1(second) wait for its read.
        def pool_block(blk, prev_pool):
            base = blk * 16 * 256
            jm = jm_bufs[1][:]
            i_pjm = nc.gpsimd.tensor_tensor(
                out=jm, in0=ahist[:, base:base + 4096],
                in1=bhist[:, base:base + 4096], op=mult)
            if prev_pool is None:
                add_dep_helper(i_pjm.ins, state["last_pool_ins_f"].ins, False,
                               "pool-order")
                add_dep_helper(i_pjm.ins, state["last_pool_ins_b"].ins, False,
                               "pool-order")
            else:
                add_dep_helper(i_pjm.ins, prev_pool.ins, False, "pool-order")
            i_l1 = nc.gpsimd.tensor_tensor(
                out=_ap(tr_pool_t[:], 0, [[16, 128], [1, 16]]),
                in0=_ap(jm, 0, [[32, 128], [1, 16]]),
                in1=_ap(jm, 16, [[32, 128], [1, 16]]),
                op=add)
            return i_l1

        def pool_tail(blk, j, prev_dve):
            # absorber: i_pt carries the Pool wait (that block's L1 write) so
            # the first tree level keeps only its DVE-self wait
            i_pt = nc.vector.tensor_copy(
                out=dve_scr[0:1, 35 + j:36 + j], in_=tr_pool_t[0:1, 0:1])
            add_dep_helper(i_pt.ins, prev_dve.ins, False, "dve-order")
            cur = _ap(tr_pool_t[:], 0, [[1, 2048]])
            tree_tail(cur, 16, blk, eng_first=i_pt)
            return i_pt

        # ---------------- epilogue: llr = ln(j0 / j1), split in k-halves ---
        # ratio first: j0/j1 = exp(llr) stays in the ACT Ln table's valid
        # input range, while raw jsum values (prenorm products) can reach
        # e^70 and fall off the table. Half A (k 0..63, jm blocks 0-3) is
        # emitted mid-epilogue so its Ln + store DMA overlap the remaining
        # blocks; half B finishes after the Pool-block tails.
        rat_t = ctx.enter_context(nc.sbuf_tensor("ratbuf", [128, 512], F32))
        rat = rat_t[:]
        llr_t2 = ctx.enter_context(nc.sbuf_tensor("llrsb", [128, 512], F32))
        llr_sb = llr_t2

        def llr_part(k0, nk, last=False):
            rcp = llr_sb[:]  # scratch for 1/j1 before Ln overwrites it
            in0 = _ap(jsum, k0 * 8, [[2, CF], [8, nk]])
            in1 = _ap(jsum, k0 * 8 + 1, [[2, CF], [8, nk]])
            rcp_h = _ap(rcp, k0, [[128, CF], [1, nk]])
            rat_h = _ap(rat, k0, [[128, CF], [1, nk]])
            nc.vector.reciprocal(out=rcp_h, in_=in1)
            nc.vector.tensor_tensor(out=rat_h, in0=in0, in1=rcp_h, op=mult)
            nc.scalar.activation(out=_ap(llr_sb[:], k0, [[128, CF], [1, nk]]),
                                 in_=rat_h,
                                 func=mybir.ActivationFunctionType.Ln)
            if last:
                # single fused store (two DMAs would land on two HW queues
                # and the exit drain can carry only one sem wait)
                src_ap = llr_sb[:].rearrange("p (f k) -> p f k", f=4)
                dst_ap = bass.AP(tensor=out_d[:].tensor, offset=0,
                                 ap=[[2048, 32], [512, 4], [128, 4], [1, 128]])
                nc.sync.dma_start(out=dst_ap, in_=src_ap)

        i_l1a = pool_block(NBLK - 2, None)

        # blocks of 16 k-steps: jm [128, 16*256]; cols k(16) f(4) b(2) m(32)
        # bhist[0] (bwd t=TLOOP-2) is the last-scheduled Pool history write;
        # Pool is in-order so waiting on it covers the fwd one too
        i_ep = nc.vector.tensor_copy(out=dve_scr[0:1, 33:34],
                                     in_=bhist[0:1, 0:1])
        prev_ep = i_ep
        for blk in range(NBLK - NPOOL):
            base = blk * 16 * 256
            jm = jm_bufs[0][:]
            i_jm = nc.vector.tensor_tensor(
                out=jm, in0=ahist[:, base:base + 4096],
                in1=bhist[:, base:base + 4096], op=mult)
            add_dep_helper(i_jm.ins, prev_ep.ins, False, "dve-order")
            prev_ep = i_jm
            tree_tail(jm, 32, blk)
            if blk == 3:
                # tail of the first Pool block, early enough that the second
                # Pool block's L1 never stalls on its tr_pool read
                prev_ep = pool_tail(NBLK - 2, 0, prev_ep)
                llr_part(0, 64)
            if blk == NBLK - NPOOL - 1:
                # k 64..95 (blocks 4-5) transform as soon as their trees land
                llr_part(64, 32)
        i_l1b = pool_block(NBLK - 1, i_l1a)
        pool_tail(NBLK - 1, 1, prev_ep)
        # only the Pool blocks' k-range remains on the final serial tail
        llr_part(96, 32, last=True)

        if dbg:
            nc.sync.dma_start(out=dbg_jsum[:], in_=jsum)
            nc.sync.dma_start(out=dbg_ah[:, 0:256], in_=ahist[:, 0:256])
            nc.sync.dma_start(out=dbg_ah[:, 256:512], in_=ahist[:, 64*256:64*256+256])
            nc.sync.dma_start(out=dbg_bh[:, 0:256], in_=bhist[:, 0:256])
            nc.sync.dma_start(out=dbg_bh[:, 256:512], in_=bhist[:, 64*256:64*256+256])
            nc.sync.dma_start(out=dbg_jm[:], in_=jm_bufs[0][:][:, 0:512])

    return nc


_ENG_SELF = {"PE": "PE_", "DVE": "DVE_", "Activation": "Activation_",
             "Pool": "Pool_", "SP": "SP_"}


def _prune_waits(nc):
    """Drop sem waits already implied, so each instruction carries <=1.

    HW structs accept one sync wait per instruction. Tile emits waits that
    are provably satisfied at issue. Vector-clock rules:
      - cross-engine sems: knowledge from transitive joins of kept waits
      - self sems (same engine): only monotone vs explicitly-waited values
        (ACT/DVE completion is not implied by issue order); PE and DMA
        queues complete in order, so own-increment knowledge counts there.
    """
    know = {}        # proc -> {sem_id: known completed value}
    waited_max = {}  # proc -> {sem_id: max explicitly waited}
    sem_total = {}   # sem_id -> running total
    hist = {}        # sem_id -> [(total_after, snapshot)]
    out_dma_sems = set()
    bad = []
    for b in nc.m.functions[0].blocks:
        for i in b.instructions:
            si = i.sync_info
            op = str(getattr(i, "opcode", type(i).__name__))
            if si is None:
                continue
            upds = [u for u in (si.on_update or [])
                    if u.sync_type == "semaphore"
                    and u.update_mode in ("sem-inc", "sem-add-imm")]
            if "DMACopy" in op and upds:
                proc = str(upds[0].ant_name)
                outs = getattr(i, "outs", None) or []
                if outs and "llr_out" in str(getattr(outs[0], "memref", "")):
                    out_dma_sems.add(upds[0].id)
            else:
                proc = getattr(i.engine, "value", str(i.engine))
            k = know.setdefault(proc, {})
            wm = waited_max.setdefault(proc, {})
            in_order = (proc == "PE" or proc == "Pool"
                        or proc.startswith("DMAHW"))
            if "Drain" in op and si.on_wait and len(si.on_wait) > 1:
                keep_d = [w for w in si.on_wait if w.id in out_dma_sems]
                # several output DMAs on one queue: keep only the max-value
                # wait per sem (the queue completes in order)
                best = {}
                for w in keep_d:
                    b = best.get(w.id)
                    if b is None or (w.wait_value or 0) > (b.wait_value or 0):
                        best[w.id] = w
                si.on_wait = list(best.values())
                continue
            skip = ("Drain" in op) or ("EventSem" in op)
            ow = list(si.on_wait or [])
            if ow and not skip:
                keep = []
                for w in ow:
                    if (w.sync_type != "semaphore"
                            or w.wait_mode != "sem-ge-imm"
                            or w.wait_value is None
                            or str(w.ant_name).startswith("barrier")):
                        keep.append(w)
                        continue
                    v = w.wait_value
                    nm = str(w.ant_name)
                    is_self = nm == proc or nm.startswith(proc + "_")
                    if is_self:
                        implied = (wm.get(w.id, -1) >= v
                                   or (in_order and k.get(w.id, 0) >= v))
                    else:
                        implied = (k.get(w.id, 0) >= v
                                   or wm.get(w.id, -1) >= v)
                    if implied:
                        continue
                    keep.append(w)
                    wm[w.id] = max(wm.get(w.id, -1), v)
                    for tot, snap in hist.get(w.id, ()):
                        if tot >= v:
                            for s2, v2 in snap.items():
                                if k.get(s2, 0) < v2:
                                    k[s2] = v2
                            break
                    if k.get(w.id, 0) < v:
                        k[w.id] = v
                if len(keep) != len(ow):
                    si.on_wait = keep
                    ow = keep
                if len(ow) > 1:
                    bad.append((i.name, op,
                                [(x.ant_name, x.wait_value) for x in ow]))
            for u in upds:
                tot = sem_total.get(u.id, 0) + (u.update_value or 0)
                sem_total[u.id] = tot
                k[u.id] = tot
                hist.setdefault(u.id, []).append((tot, dict(k)))
    if bad:
        raise RuntimeError(f"{len(bad)} insts still multi-wait: {bad[:8]}")
    return nc


def _get_nc():
    if "nc" not in _NC_CACHE:
        _NC_CACHE["nc"] = _prune_waits(build_nc())
    return _NC_CACHE["nc"]


# ---------------- host-side layout ----------------
def _prep_core(llr_ch_c, llr_a_c):
    """llr_ch_c [32, 4096], llr_a_c [32, 2048] -> llr_t [12, TL*128] bf16."""
    lc = np.zeros((B_CORE, T + 2 * L, 2), np.float32)
    lc[:, L:L + T] = llr_ch_c.reshape(B_CORE, T, 2)
    la = np.full((B_CORE, T + 2 * L), PAD_A, np.float32)
    la[:, L:L + T] = llr_a_c
    # windows [B, C, TL, comp]
    idx = (np.arange(C)[:, None] * S + np.arange(TL)[None, :])  # [C, TL]
    w = np.stack([la[:, idx], lc[:, idx, 0], lc[:, idx, 1]], -1)  # [B, C, TL, 3]
    # chunk c = g*4+f ; row = cw*4+g ; llr_t[f*3+comp, tau*128+row]
    w = w.reshape(B_CORE, 4, 4, TL, 3)            # [cw, g, f, tau, comp]
    w = w.transpose(2, 4, 3, 0, 1)                # [f, comp, tau, cw, g]
    return np.ascontiguousarray(
        w.reshape(12, TL * 128)).astype(ml_dtypes.bfloat16)


def _run(llr_ch, llr_a, trace=False):
    nc = _get_nc()
    in_maps = []
    for core in range(N_CORES):
        sl = slice(core * B_CORE, (core + 1) * B_CORE)
        in_maps.append({
            "llr_t": _prep_core(np.asarray(llr_ch[sl], np.float32),
                                np.asarray(llr_a[sl], np.float32)),
            "sign": SIGN_BD,
        })
    res = run_bass_kernel_spmd(nc, in_maps, core_ids=list(range(N_CORES)),
                               trace=trace)
    out = np.concatenate([r["llr_out"] for r in res.results], 0)
    return out.astype(np.float32), res


def kernel(llr_ch, llr_a):
    out, _ = _run(llr_ch, llr_a, trace=False)
    return out

